# revision 24
# baseline (speedup 1.0000x reference)
"""Trainium2 Bass kernel for BertInfiniSelfAttention — v3 design.

Sharding (8 cores): core c = (batch b = c//4, kv-quarter q = c%4).
Each core owns batch b and kv heads {3q, 3q+1, 3q+2} (KVL = 2304 of the
9216 concatenated kv positions), computes the full Q projection for its
batch, the K/V projections + memory matmuls for its 3 kv heads, then
flash-style partial attention for all 12 q heads against its local KV.
Host sums partial (numerator, denominator) over the 4 kv-quarters per
batch and divides.

v3: the ctx (probs @ V) matmuls run in fp8e4 DoubleRow mode — two kv
tiles (K=256) per pass at 1 column/cycle, 2x the fp16 rate.  Direct fp8
quantization of probs/values would exceed the error budget (softmax
averages 9216 near-uniform weights, so 3.6% rms fp8 noise survives into
the output), so the kernel uses a shift trick:

    ctx = sum_kv pr*vc = (sum_kv vc) + sum_kv (pr-1)*vc

The first term (vcsum) is computed exactly in fp16 via host-side row
sums of mvt (vcsum = mvtsum @ v, 18 N=1 matmuls) and added at flush
time; only the small residual pr' = exp(s)-1 (|pr'| ~ 0.17) and vc are
fp8, so quantization noise lands on a 6x smaller term.  The denominator
row works the same way: row 64 accumulates em*pr', and emsum (host
constant) is added at flush.

pr' production: qT is pre-scaled by 1/512 so scores psum holds s/64.
DVE tiles use a custom op (1+x)^64 - 1 (8 ALU stages); Act tiles
compute exp(64x) into fp16 staging and the otherwise-idle GpSimd engine
applies -1 with an fp8 cast.

Startup: wq/hT arrive in kt-major layout and are DMAed in 2-chunk
groups with per-group semaphores so the Q projection starts on group 0
(~12us) instead of after the full 2.4 MB (~20us).
"""

import numpy as np

B, S, H, NH, D = 2, 768, 768, 12, 64
P = 128
NCORES = 8
NQUART = 4              # kv quarters
HK = 3                  # kv heads per core
KVL = HK * S            # 2304 local kv
NKT = KVL // P          # 18 kv tiles
ACT_LIST = (0, 2, 4, 6, 8, 10, 12, 14, 16, 17)   # fp16 path (Act exp)
DVE_LIST = (1, 3, 5, 7, 9, 11, 13, 15)           # fp8 path (DVE expm1)
NACT = len(ACT_LIST)    # 10
NDVE = len(DVE_LIST)    # 8
NPR8 = NDVE // 2        # 4 fp8 DoubleRow kv-tile pairs
NT = S // P             # 6 s/H tiles
DP1 = D + 1
VSTR = 80               # vca8 slab stride (DP1 padded to a mult of 16)
NPAIR = NH // 2         # 6 q-head pairs
SH = S // 2             # 384, flush s-half
QSC = 1.0 / 512.0       # qT pre-scale: 1/sqrt(D)/64 folded into qT

ACT_SLOT = {t: i for i, t in enumerate(ACT_LIST)}
DVE_SLOT = {t: i for i, t in enumerate(DVE_LIST)}

_PROGRAM = None
TRACE = False
LAST_RESULTS = None


def _bank_pieces(lo, hi):
    """Split [lo,hi) free-dim range at 512-fp32 PSUM bank boundaries."""
    out = []
    while lo < hi:
        nxt = min(hi, (lo // 512 + 1) * 512)
        out.append((lo, nxt))
        lo = nxt
    return out


def _expm1_ref(in0, in1, s0, s1, imm2):
    t = (in0 + 1.0).astype(np.float32)
    for _ in range(6):
        t = t * t
    return t - 1.0


def _make_expm1_op():
    from concourse import dve_ops as DO
    from concourse.dve_spec import Spec, Src0, One, sq

    for o in DO.OPS:
        if o.name == "EXPM1_SQ64_ANT":
            return o
    body = sq(sq(sq(sq(sq(sq(Src0 + One)))))) - One
    op = DO.DveOp(
        "EXPM1_SQ64_ANT",
        Spec(body=body, reference=_expm1_ref),
        subdim=False,
        uops_sha={"v3": "0d629377a67c4031", "v4": "a8d0e57c9f1ce618"},
    )
    DO.OPS.append(op)
    DO._SUB_OPCODE_FOR_NAME[op.name] = DO._CUSTOM_DVE_ROW_BASE + len(DO.OPS) - 1
    return op


def _build_program():
    from contextlib import ExitStack
    from itertools import zip_longest

    import concourse.bacc as bacc
    import concourse.mybir as mybir
    import concourse.tile as tile

    expm1_op = _make_expm1_op()

    F32 = mybir.dt.float32
    F16 = mybir.dt.float16
    F8 = mybir.dt.float8e4
    EXP = mybir.ActivationFunctionType.Exp
    IDT = mybir.ActivationFunctionType.Identity
    DR = mybir.MatmulPerfMode.DoubleRow
    ADD = mybir.AluOpType.add
    MULT = mybir.AluOpType.mult

    nc = bacc.Bacc("TRN2", target_bir_lowering=False, debug=False,
                   num_devices=NCORES)

    # hT / wq come in kt-major layout ([p, kt, c], pre-shuffled on the host)
    # so chunk-group DMAs are contiguous with 3 KiB per partition line
    hT = nc.declare_dram_parameter("hT", [P, NT * S], F16, isOutput=False)
    wq = nc.declare_dram_parameter("wq", [P, NT * H], F16, isOutput=False)
    # bq_d cols 0:NPAIR = bq (pair layout); NPAIR:2*NPAIR = bq/512
    bq_d = nc.declare_dram_parameter("bq_d", [P, 2 * NPAIR], F32,
                                     isOutput=False)
    wkv = nc.declare_dram_parameter("wkv", [H, 6 * D], F16, isOutput=False)
    bkv_d = nc.declare_dram_parameter("bkv_d", [1, 6 * D], F16, isOutput=False)
    mkt = nc.declare_dram_parameter("mkt", [S, KVL], F16, isOutput=False)
    mvt = nc.declare_dram_parameter("mvt", [S, KVL], F16, isOutput=False)
    em8_d = nc.declare_dram_parameter("em8_d", [P, NDVE], F8, isOutput=False)
    em16_d = nc.declare_dram_parameter("em16_d", [P, NACT], F16,
                                       isOutput=False)
    # mvts_d[p, i*NT+st] = sum_{kv in DVE tile i} mvt[st*128+p, kv]  (fp16)
    mvts_d = nc.declare_dram_parameter("mvts_d", [P, NDVE * NT], F16,
                                       isOutput=False)
    emsum_d = nc.declare_dram_parameter("emsum_d", [1, 1], F32,
                                        isOutput=False)
    ones_d = nc.declare_dram_parameter("ones_d", [1, S], F16, isOutput=False)
    out_d = nc.declare_dram_parameter("out_d", [NH, DP1, S], F32, isOutput=True)

    with tile.TileContext(nc) as tc, ExitStack() as ctx:
        const = ctx.enter_context(tc.tile_pool(name="const", bufs=1))

        qT = const.tile([P, NPAIR * S], F16, name="qT")
        kcT = const.tile([P, KVL], F16, name="kcT")
        # vca16: fp16 vc+em for the Act-path tiles (ACT_LIST order)
        vca16 = const.tile([P, NACT * DP1], F16, name="vca16")
        # vca8[p, j, s, c]: DVE kv-tile pair j, slab s, c = 64 vc cols +
        # em col, padded to VSTR=80 so the DoubleRow weights AP slab step
        # is a multiple of 16 (s3_lw dual-fp8 ISA restriction)
        vca8 = const.tile([P, NPR8 * 2 * VSTR], F8, name="vca8")
        ones = const.tile([1, S], F16, name="ones")
        bq_s = const.tile([P, 2 * NPAIR], F32, name="bq_s")
        bkv_s = const.tile([1, 6 * D], F16, name="bkv_s")
        mvts_s = const.tile([P, NDVE * NT], F16, name="mvts_s")
        vcsum_col = const.tile([DP1, 1], F32, name="vcsum_col")
        act_warm = const.tile([1, 8], F16, name="act_warm")
        act_in = const.tile([1, 8], F16, name="act_in")

        nc.gpsimd.dma_start(ones[:], ones_d[:])
        nc.gpsimd.dma_start(bq_s[:], bq_d[:])
        nc.gpsimd.dma_start(bkv_s[:], bkv_d[:])
        nc.gpsimd.dma_start(mvts_s[:], mvts_d[:])
        # prefill the em (denominator) columns of vca8/vca16 from DRAM
        nc.gpsimd.dma_start(
            vca8[:].rearrange("p (j s c) -> p j s c", s=2, c=VSTR)[:, :, :,
                                                                  D:DP1],
            em8_d[:].rearrange("p (j s c) -> p j s c", s=2, c=1))
        nc.gpsimd.dma_start(
            vca16[:].rearrange("p (t c) -> p t c", c=DP1)[:, :, D:DP1],
            em16_d[:].rearrange("p (t c) -> p t c", c=1))
        # emsum goes into the denominator row of the vcsum column
        nc.sync.dma_start(vcsum_col[D:DP1, :], emsum_d[:])
        # memset-sourced input for the exp-table warm: no DMA dependency, so
        # the scalar engine reaches its hT dma_starts without stalling
        nc.vector.memset(act_in[:], 1.0)

        # ---- long-lived inputs (kv3 + mvt live until vc is done) ----
        iov = ctx.enter_context(tc.tile_pool(name="iov", bufs=1))
        kv3 = iov.tile([P, NT * 6 * D], F16, name="kv3")
        mvt_s = iov.tile([P, NT * KVL], F16, name="mvt_s")

        with tc.tile_pool(name="iok", bufs=1) as iok:
            mkt_s = iok.tile([P, NT * KVL], F16, name="mkt_s")

            # ---- Phase A ----
            with tc.tile_pool(name="ioa", bufs=1) as ioa:
                wq_s = ioa.tile([P, NT * H], F16, name="wq_s")
                hT_s = ioa.tile([P, NT * S], F16, name="hT_s")
                wkv_s = ioa.tile([P, NT * 6 * D], F16, name="wkv_s")

                # DMA queues: sync = wq, wkv then outputs; scalar = hT
                # only (keeps the ACT instruction stream clean for phase-C
                # exps); gpsimd = consts, mkt, then mvt.  wq/hT arrive in
                # kt-major layout, issued as 2-chunk groups: contiguous
                # 3 KiB partition lines, with per-group completion
                # semaphores so the kt-chunked Q proj starts on group 0.
                for g in range(NT // 2):
                    nc.sync.dma_start(wq_s[:, 2 * g * H:2 * (g + 1) * H],
                                      wq[:, 2 * g * H:2 * (g + 1) * H])
                    nc.scalar.dma_start(hT_s[:, 2 * g * S:2 * (g + 1) * S],
                                        hT[:, 2 * g * S:2 * (g + 1) * S])
                for kt in range(NT):
                    nc.sync.dma_start(wkv_s[:, kt * 6 * D:(kt + 1) * 6 * D],
                                      wkv[kt * P:(kt + 1) * P, :])
                # warm the exp table (ACT_TABLE_LOAD ~1.3us) during phase A
                nc.scalar.activation(act_warm[:], act_in[:], EXP, scale=64.0)
                for h in range(HK):
                    for st in range(NT):
                        nc.gpsimd.dma_start(
                            mkt_s[:, st * KVL + h * S: st * KVL + (h + 1) * S],
                            mkt[st * P:(st + 1) * P, h * S:(h + 1) * S])
                for h in range(HK):
                    for st in range(NT):
                        nc.gpsimd.dma_start(
                            mvt_s[:, st * KVL + h * S: st * KVL + (h + 1) * S],
                            mvt[st * P:(st + 1) * P, h * S:(h + 1) * S])

                # Q projection (pair-outer) + K/V projection.  qT is scaled
                # by 1/512 (= softmax 1/8 fused with the exp approximation's
                # 1/64) so phase C's DVE expm1 op needs no multiply stage.
                with tc.tile_pool(name="aps", bufs=2, space="PSUM") as aps:
                    for t in range(NPAIR):
                        q_ps = aps.tile([P, S], F32, name="q_ps", tag="q_ps")
                        for lo, hi in _bank_pieces(0, S):
                            for kt in range(NT):
                                nc.tensor.matmul(
                                    q_ps[:, lo:hi],
                                    wq_s[:, kt * H + t * P: kt * H + (t + 1) * P],
                                    hT_s[:, kt * S + lo: kt * S + hi],
                                    start=(kt == 0), stop=(kt == NT - 1))
                        if t % 2 == 0:
                            nc.vector.tensor_scalar(
                                qT[:, t * S:(t + 1) * S], q_ps[:],
                                bq_s[:, t:t + 1], QSC, op0=ADD, op1=MULT)
                        else:
                            nc.scalar.activation(
                                qT[:, t * S:(t + 1) * S], q_ps[:], IDT,
                                bias=bq_s[:, NPAIR + t:NPAIR + t + 1],
                                scale=QSC)

                    for st in range(NT):
                        kv_ps = aps.tile([P, 6 * D], F32, name="kv_ps",
                                         tag="kv_ps")
                        for kt in range(NT):
                            nc.tensor.matmul(
                                kv_ps[:],
                                hT_s[:, kt * S + st * P: kt * S + (st + 1) * P],
                                wkv_s[:, kt * 6 * D:(kt + 1) * 6 * D],
                                start=(kt == 0), stop=False)
                        nc.tensor.matmul(kv_ps[:], ones[:, 0:P], bkv_s[:],
                                         start=False, stop=True)
                        nc.vector.tensor_copy(
                            kv3[:, st * 6 * D:(st + 1) * 6 * D], kv_ps[:])

            # ---- Phase B: kc (kv-duplicated halves, concurrent col pairs)
            with tc.tile_pool(name="kcps", bufs=2, space="PSUM") as kcps:
                for h in range(HK):
                    kc_ps = kcps.tile([P, S], F32, name="kc_ps", tag="kc_ps")
                    for lo, hi in _bank_pieces(0, S):
                        for st in range(NT):
                            lhsT = kv3[:, st * 6 * D + h * D:
                                       st * 6 * D + (h + 1) * D]
                            rhs = mkt_s[:, st * KVL + h * S + lo:
                                        st * KVL + h * S + hi]
                            nc.tensor.matmul(
                                kc_ps[0:D, lo:hi], lhsT, rhs,
                                start=(st == 0), stop=(st == NT - 1))
                            nc.tensor.matmul(
                                kc_ps[D:P, lo:hi], lhsT, rhs,
                                start=(st == 0), stop=(st == NT - 1),
                                tile_position=(0, D))
                    nc.vector.tensor_copy(kcT[:, h * S:(h + 1) * S], kc_ps[:])

                # vcsum[d] = sum_{kv in DVE tiles} vc_em[kv, d]
                #          = sum_st mvts[st] @ v[st]  per DVE tile
                # (exact-in-fp16 correction term for the pr-1 shift trick)
                with tc.tile_pool(name="vsps", bufs=1, space="PSUM") as vsps:
                    vs_ps = vsps.tile([D, 1], F32, name="vs_ps")
                    n = 0
                    for i, t in enumerate(DVE_LIST):
                        h = t // NT
                        for st in range(NT):
                            nc.tensor.matmul(
                                vs_ps[:],
                                kv3[:, st * 6 * D + (HK + h) * D:
                                    st * 6 * D + (HK + h + 1) * D],
                                mvts_s[:, i * NT + st: i * NT + st + 1],
                                start=(n == 0), stop=(n == NDVE * NT - 1))
                            n += 1
                    nc.vector.tensor_copy(vcsum_col[0:D, :], vs_ps[:])

        # ---- Phase C ----
        # Act-path pr tiles: [p, head(2)*S] fp16 holding pr = exp(s);
        # DVE-path pair tiles: [p, slab(2), head(2)*S] fp8 with pr'=exp-1
        prp16 = ctx.enter_context(tc.tile_pool(name="prp16", bufs=22))
        prp8 = ctx.enter_context(tc.tile_pool(name="prp8", bufs=10))
        scps = ctx.enter_context(tc.tile_pool(name="scps", bufs=2,
                                              space="PSUM"))
        stg = ctx.enter_context(tc.tile_pool(name="stg", bufs=4))

        def emit_scores(p, t, dst):
            """Concurrent row-tiled pair: even head -> cols 0:768, odd
            head -> cols 768:1536 of a [128, 1536] psum tile; exp into
            the fp16 pr tile (Act tiles) or exp-1 into an fp8 pair-tile
            slab (DVE tiles)."""
            sc = scps.tile([P, 2 * S], F32, name="sc", tag="sc")
            kc_lo = kcT[0:D, t * P:(t + 1) * P]
            kc_hi = kcT[D:P, t * P:(t + 1) * P]
            for pa, pb in zip_longest(_bank_pieces(0, S),
                                      _bank_pieces(S, 2 * S)):
                if pa is not None:
                    lo, hi = pa
                    nc.tensor.matmul(sc[:, lo:hi], kc_lo,
                                     qT[0:D, p * S + lo: p * S + hi],
                                     start=True, stop=True)
                if pb is not None:
                    lob, hib = pb
                    nc.tensor.matmul(
                        sc[:, lob:hib], kc_hi,
                        qT[D:P, p * S + lob - S: p * S + hib - S],
                        start=True, stop=True)
            if t in ACT_SLOT:
                nc.scalar.activation(dst, sc[:], EXP, scale=64.0)
            else:
                nc.vector._custom_dve(expm1_op, out=dst, in0=sc[:])

        def emit_ctx16(a, pr16, half, ps, first, last):
            """fp16 ctx for Act-path tile slot a (unshifted pr)."""
            lhsT = vca16[:, a * DP1:(a + 1) * DP1]
            for lo, hi in _bank_pieces(0, S):
                nc.tensor.matmul(
                    ps[:, lo:hi], lhsT,
                    pr16[:, half * S + lo: half * S + hi],
                    start=first, stop=last)

        def emit_ctx8(j, pair, half, ps, first, last):
            """fp8 DoubleRow: DVE kv-tile pair j (K=256) in one pass."""
            lhsT = vca8[:, j * 2 * VSTR:(j + 1) * 2 * VSTR].rearrange(
                "p (s c) -> p s c", c=VSTR)[:, :, 0:DP1]
            rhs3 = pair[:].rearrange("p (s x) -> p s x", s=2)
            for lo, hi in _bank_pieces(0, S):
                nc.tensor.matmul(
                    ps[:, lo:hi], lhsT,
                    rhs3[:, :, half * S + lo: half * S + hi],
                    start=first, stop=last, perf_mode=DR)

        def flush_ctx(ps, head):
            # add the shift-trick correction (vcsum / emsum) while copying
            # psum -> sbuf, split across the two exp engines
            st_t = stg.tile([DP1, S], F32, name="st_t", tag="st")
            nc.vector.tensor_scalar_add(st_t[:, 0:SH], ps[:, 0:SH],
                                        vcsum_col[:, 0:1])
            nc.scalar.activation(st_t[:, SH:S], ps[:, SH:S], IDT,
                                 bias=vcsum_col[:, 0:1])
            nc.sync.dma_start(out_d[head], st_t[:])

        def emit_vc(t):
            h = t // NT
            vc_ps = vcps.tile([P, D], F32, name="vc_ps", tag="vc")
            for st in range(NT):
                nc.tensor.matmul(
                    vc_ps[:],
                    mvt_s[:, st * KVL + t * P: st * KVL + (t + 1) * P],
                    kv3[:, st * 6 * D + (HK + h) * D:
                        st * 6 * D + (HK + h + 1) * D],
                    start=(st == 0), stop=(st == NT - 1))
            if t in ACT_SLOT:
                a = ACT_SLOT[t]
                nc.vector.tensor_copy(
                    vca16[:, a * DP1: a * DP1 + D], vc_ps[:])
            else:
                i = DVE_SLOT[t]
                nc.vector.tensor_copy(
                    vca8[:, (i // 2) * 2 * VSTR + (i % 2) * VSTR:
                         (i // 2) * 2 * VSTR + (i % 2) * VSTR + D],
                    vc_ps[:])

        def alloc_pr(t, store):
            """Allocate/locate the exp destination for kv tile t."""
            if t in ACT_SLOT:
                pr16 = prp16.tile([P, 2 * S], F16, name="pr16", tag="pr16")
                store["act"].append(pr16)
                return pr16[:]
            i = DVE_SLOT[t]
            if i % 2 == 0:
                store["dve"].append(
                    prp8.tile([P, 4 * S], F8, name="pr8", tag="pr8"))
            pair = store["dve"][-1]
            return pair[:, (i % 2) * 2 * S:(i % 2 + 1) * 2 * S]

        # ctx work items per head-half: 10 fp16 groups + 4 fp8-DR groups,
        # packed into the FIRST 8 slots of the half-sweep with the flush at
        # slot 7 -- slots 8/17 are scores-only, giving the psum accumulator
        # WAW dependency ~2 slots of slack before the next chain starts
        NITEM = NACT + NPR8

        def emit_ctx_items(slot, prev, half, ps, head_flush):
            if slot > 7:
                return
            for k in range(NITEM * slot // 8, NITEM * (slot + 1) // 8):
                if k < NACT:
                    emit_ctx16(k, prev["act"][k], half, ps,
                               first=(k == 0), last=(k == NITEM - 1))
                else:
                    emit_ctx8(k - NACT, prev["dve"][k - NACT], half, ps,
                              first=(k == 0), last=(k == NITEM - 1))
            if slot == 7:
                flush_ctx(ps, head_flush)

        # sweep 0: scores/exp for pair 0 with the vc matmuls interleaved
        # (PE filler under the engine-paced exp window)
        prev = {"act": [], "dve": []}
        with tc.tile_pool(name="vcps", bufs=2, space="PSUM") as vcps:
            for t in range(NKT):
                emit_scores(0, t, alloc_pr(t, prev))
                emit_vc(t)

        # sweeps 1..6: scores/exp for pair p + deferred ctx chains for
        # pair p-1 (A then B) sharing one psum accumulator
        with tc.tile_pool(name="ctxps", bufs=1, space="PSUM") as ctxps:
            for p in range(1, NPAIR + 1):
                cur = {"act": [], "dve": []}
                ps = None
                for t in range(NKT):
                    dst = alloc_pr(t, cur) if p < NPAIR else None
                    # chain-start slots (0,1 and 9,10): ctx items FIRST so
                    # the in-order PE has fill work while the previous
                    # sweep's exp tail drains the scores psum buffers
                    if t < 9:   # ctx A
                        if t == 0:
                            ps = ctxps.tile([DP1, S], F32, name="ctx",
                                            tag="ctx")
                        emit_ctx_items(t, prev, 0, ps, 2 * (p - 1))
                    else:       # ctx B
                        tb = t - 9
                        if tb == 0:
                            ps = ctxps.tile([DP1, S], F32, name="ctx",
                                            tag="ctx")
                        emit_ctx_items(tb, prev, 1, ps, 2 * (p - 1) + 1)
                    if p < NPAIR:
                        emit_scores(p, t, dst)
                prev = cur

    nc.compile()
    return nc


def _get_program():
    global _PROGRAM
    if _PROGRAM is None:
        _PROGRAM = _build_program()
    return _PROGRAM


def kernel(hidden_states, attention_mask, Wq, bq, Wk, bk, Wv, bv, gate,
           mem_keys, mem_values):
    import ml_dtypes
    from concourse.bass_utils import run_bass_kernel_spmd

    global LAST_RESULTS

    f32, f16 = np.float32, np.float16
    f8 = ml_dtypes.float8_e4m3
    hidden_states = np.asarray(hidden_states, f32)
    attention_mask = np.asarray(attention_mask, f32)
    Wq = np.asarray(Wq, f32)
    bq = np.asarray(bq, f32)
    Wk = np.asarray(Wk, f32)
    bk = np.asarray(bk, f32)
    Wv = np.asarray(Wv, f32)
    bv = np.asarray(bv, f32)
    gate = np.asarray(gate, f32)
    mem_keys = np.asarray(mem_keys, f32)
    mem_values = np.asarray(mem_values, f32)

    # kt-major device layouts: x_kt[p, kt*C + c] = x[kt*128 + p, c]
    hT16 = [np.ascontiguousarray(
                hidden_states[b].T.reshape(NT, P, S).transpose(1, 0, 2)
                .reshape(P, NT * S)).astype(f16)
            for b in range(B)]
    wq16 = np.ascontiguousarray(
        Wq.reshape(NT, P, H).transpose(1, 0, 2).reshape(P, NT * H)
    ).astype(f16)
    bq_pair = np.ascontiguousarray(
        bq.reshape(NPAIR, 2, D).transpose(1, 2, 0).reshape(P, NPAIR))
    bq_dev = np.concatenate([bq_pair, bq_pair * QSC], axis=1)
    em_full = np.exp(attention_mask.reshape(B, NH * S)).astype(f32)
    ones_dev = np.ones((1, S), f16)

    in_maps = []
    for c in range(NCORES):
        b, quart = c // NQUART, c % NQUART
        heads = [HK * quart + j for j in range(HK)]
        wkv_c = np.concatenate(
            [Wk[:, h * D:(h + 1) * D] for h in heads]
            + [Wv[:, h * D:(h + 1) * D] for h in heads], axis=1)
        bkv_c = np.concatenate(
            [bk[h * D:(h + 1) * D] for h in heads]
            + [bv[h * D:(h + 1) * D] for h in heads])[None, :]
        # mkt[s, h_local*768+kv] = mem_keys[heads[h_local], kv, s]
        mkt_c = mem_keys[heads].transpose(2, 0, 1).reshape(S, KVL)
        em_c = em_full[b, quart * KVL:(quart + 1) * KVL]
        mvt_c = (mem_values[heads].transpose(2, 0, 1).reshape(S, KVL)
                 * em_c[None, :])
        em_tiles = em_c.reshape(NKT, P)                      # [t, p]
        em16_dev = np.ascontiguousarray(em_tiles[list(ACT_LIST)].T)
        em8_dev = np.ascontiguousarray(em_tiles[list(DVE_LIST)].T)
        # per-DVE-tile row sums of mvt (em-weighted), kt-chunked:
        # mvts_dev[p, i*NT+st] = sum_{kv in DVE tile i} mvt[st*128+p, kv]
        msum = mvt_c.reshape(S, NKT, P).sum(axis=2)          # [S, NKT]
        msum = msum[:, list(DVE_LIST)]                       # [S, NDVE]
        mvts_dev = np.ascontiguousarray(
            msum.reshape(NT, P, NDVE).transpose(1, 2, 0).reshape(P,
                                                                 NDVE * NT))
        emsum_dve = em_tiles[list(DVE_LIST)].sum()
        in_maps.append({
            "hT": hT16[b],
            "wq": wq16,
            "bq_d": bq_dev,
            "wkv": np.ascontiguousarray(wkv_c).astype(f16),
            "bkv_d": np.ascontiguousarray(bkv_c).astype(f16),
            "mkt": np.ascontiguousarray(mkt_c).astype(f16),
            "mvt": np.ascontiguousarray(mvt_c).astype(f16),
            "em8_d": em8_dev.astype(f8),
            "em16_d": em16_dev.astype(f16),
            "mvts_d": mvts_dev.astype(f16),
            "emsum_d": np.array([[emsum_dve]], f32),
            "ones_d": ones_dev,
        })

    nc = _get_program()
    res = run_bass_kernel_spmd(nc, in_maps, core_ids=list(range(NCORES)),
                               trace=TRACE)
    LAST_RESULTS = res

    out = np.empty((B, S, NH, D), f32)
    for b in range(B):
        parts = res.results[b * NQUART]["out_d"].astype(f32).copy()
        for c in range(b * NQUART + 1, (b + 1) * NQUART):
            parts += res.results[c]["out_d"]
        num = parts[:, :D, :]                     # [12, 64, 768]
        den = parts[:, D, :]                      # [12, 768]
        ctxT = num / den[:, None, :]
        out[b] = ctxT.transpose(2, 0, 1)          # [768, 12, 64]
    g = (1.0 / (1.0 + np.exp(-gate))).reshape(1, 1, NH, 1)
    return (g * out).astype(f32)


# revision 25
# speedup vs baseline: 1.0113x; 1.0113x over previous
"""Trainium2 Bass kernel for BertInfiniSelfAttention — v3 design.

Sharding (8 cores): core c = (batch b = c//4, kv-quarter q = c%4).
Each core owns batch b and kv heads {3q, 3q+1, 3q+2} (KVL = 2304 of the
9216 concatenated kv positions), computes the full Q projection for its
batch, the K/V projections + memory matmuls for its 3 kv heads, then
flash-style partial attention for all 12 q heads against its local KV.
Host sums partial (numerator, denominator) over the 4 kv-quarters per
batch and divides.

v3: the ctx (probs @ V) matmuls run in fp8e4 DoubleRow mode — two kv
tiles (K=256) per pass at 1 column/cycle, 2x the fp16 rate.  Direct fp8
quantization of probs/values would exceed the error budget (softmax
averages 9216 near-uniform weights, so 3.6% rms fp8 noise survives into
the output), so the kernel uses a shift trick:

    ctx = sum_kv pr*vc = (sum_kv vc) + sum_kv (pr-1)*vc

The first term (vcsum) is computed exactly in fp16 via host-side row
sums of mvt (vcsum = mvtsum @ v, 18 N=1 matmuls) and added at flush
time; only the small residual pr' = exp(s)-1 (|pr'| ~ 0.17) and vc are
fp8, so quantization noise lands on a 6x smaller term.  The denominator
row works the same way: row 64 accumulates em*pr', and emsum (host
constant) is added at flush.

pr' production: qT is pre-scaled by 1/512 so scores psum holds s/64.
DVE tiles use a custom op (1+x)^64 - 1 (8 ALU stages); Act tiles
compute exp(64x) into fp16 staging and the otherwise-idle GpSimd engine
applies -1 with an fp8 cast.

Startup: wq/hT arrive in kt-major layout and are DMAed in 2-chunk
groups with per-group semaphores so the Q projection starts on group 0
(~12us) instead of after the full 2.4 MB (~20us).
"""

import numpy as np

B, S, H, NH, D = 2, 768, 768, 12, 64
P = 128
NCORES = 8
NQUART = 4              # kv quarters
HK = 3                  # kv heads per core
KVL = HK * S            # 2304 local kv
NKT = KVL // P          # 18 kv tiles
ACT_LIST = (0, 2, 4, 6, 8, 10, 12, 14, 16, 17)   # fp16 path (Act exp)
DVE_LIST = (1, 3, 5, 7, 9, 11, 13, 15)           # fp8 path (DVE expm1)
NACT = len(ACT_LIST)    # 10
NDVE = len(DVE_LIST)    # 8
NPR8 = NDVE // 2        # 4 fp8 DoubleRow kv-tile pairs
NT = S // P             # 6 s/H tiles
DP1 = D + 1
VSTR = 80               # vca8 slab stride (DP1 padded to a mult of 16)
NPAIR = NH // 2         # 6 q-head pairs
SH = S // 2             # 384, flush s-half
QSC = 1.0 / 512.0       # qT pre-scale: 1/sqrt(D)/64 folded into qT

ACT_SLOT = {t: i for i, t in enumerate(ACT_LIST)}
DVE_SLOT = {t: i for i, t in enumerate(DVE_LIST)}

_PROGRAM = None
TRACE = False
LAST_RESULTS = None


def _bank_pieces(lo, hi):
    """Split [lo,hi) free-dim range at 512-fp32 PSUM bank boundaries."""
    out = []
    while lo < hi:
        nxt = min(hi, (lo // 512 + 1) * 512)
        out.append((lo, nxt))
        lo = nxt
    return out


def _expm1_ref(in0, in1, s0, s1, imm2):
    t = (in0 + 1.0).astype(np.float32)
    for _ in range(6):
        t = t * t
    return t - 1.0


def _make_expm1_op():
    from concourse import dve_ops as DO
    from concourse.dve_spec import Spec, Src0, One, sq

    for o in DO.OPS:
        if o.name == "EXPM1_SQ64_ANT":
            return o
    body = sq(sq(sq(sq(sq(sq(Src0 + One)))))) - One
    op = DO.DveOp(
        "EXPM1_SQ64_ANT",
        Spec(body=body, reference=_expm1_ref),
        subdim=False,
        uops_sha={"v3": "0d629377a67c4031", "v4": "a8d0e57c9f1ce618"},
    )
    DO.OPS.append(op)
    DO._SUB_OPCODE_FOR_NAME[op.name] = DO._CUSTOM_DVE_ROW_BASE + len(DO.OPS) - 1
    return op


def _build_program():
    from contextlib import ExitStack
    from itertools import zip_longest

    import concourse.bacc as bacc
    import concourse.mybir as mybir
    import concourse.tile as tile

    expm1_op = _make_expm1_op()

    F32 = mybir.dt.float32
    F16 = mybir.dt.float16
    F8 = mybir.dt.float8e4
    EXP = mybir.ActivationFunctionType.Exp
    IDT = mybir.ActivationFunctionType.Identity
    DR = mybir.MatmulPerfMode.DoubleRow
    ADD = mybir.AluOpType.add
    MULT = mybir.AluOpType.mult

    nc = bacc.Bacc("TRN2", target_bir_lowering=False, debug=False,
                   num_devices=NCORES)

    # hT / wq come in kt-major layout ([p, kt, c], pre-shuffled on the host)
    # so chunk-group DMAs are contiguous with 3 KiB per partition line
    hT = nc.declare_dram_parameter("hT", [P, NT * S], F16, isOutput=False)
    wq = nc.declare_dram_parameter("wq", [P, NT * H], F16, isOutput=False)
    # bq_d cols 0:NPAIR = bq (pair layout); NPAIR:2*NPAIR = bq/512
    bq_d = nc.declare_dram_parameter("bq_d", [P, 2 * NPAIR], F32,
                                     isOutput=False)
    wkv = nc.declare_dram_parameter("wkv", [H, 6 * D], F16, isOutput=False)
    bkv_d = nc.declare_dram_parameter("bkv_d", [1, 6 * D], F16, isOutput=False)
    mkt = nc.declare_dram_parameter("mkt", [S, KVL], F16, isOutput=False)
    mvt = nc.declare_dram_parameter("mvt", [S, KVL], F16, isOutput=False)
    em8_d = nc.declare_dram_parameter("em8_d", [P, NDVE], F8, isOutput=False)
    em16_d = nc.declare_dram_parameter("em16_d", [P, NACT], F16,
                                       isOutput=False)
    # mvts_d[p, i*NT+st] = sum_{kv in DVE tile i} mvt[st*128+p, kv]  (fp16)
    mvts_d = nc.declare_dram_parameter("mvts_d", [P, NDVE * NT], F16,
                                       isOutput=False)
    emsum_d = nc.declare_dram_parameter("emsum_d", [1, 1], F32,
                                        isOutput=False)
    ones_d = nc.declare_dram_parameter("ones_d", [1, S], F16, isOutput=False)
    out_d = nc.declare_dram_parameter("out_d", [NH, DP1, S], F32, isOutput=True)

    with tile.TileContext(nc) as tc, ExitStack() as ctx:
        const = ctx.enter_context(tc.tile_pool(name="const", bufs=1))

        qT = const.tile([P, NPAIR * S], F16, name="qT")
        kcT = const.tile([P, KVL], F16, name="kcT")
        # vca16: fp16 vc+em for the Act-path tiles (ACT_LIST order)
        vca16 = const.tile([P, NACT * DP1], F16, name="vca16")
        # vca8[p, j, s, c]: DVE kv-tile pair j, slab s, c = 64 vc cols +
        # em col, padded to VSTR=80 so the DoubleRow weights AP slab step
        # is a multiple of 16 (s3_lw dual-fp8 ISA restriction)
        vca8 = const.tile([P, NPR8 * 2 * VSTR], F8, name="vca8")
        ones = const.tile([1, S], F16, name="ones")
        bq_s = const.tile([P, 2 * NPAIR], F32, name="bq_s")
        bkv_s = const.tile([1, 6 * D], F16, name="bkv_s")
        mvts_s = const.tile([P, NDVE * NT], F16, name="mvts_s")
        vcsum_col = const.tile([DP1, 1], F32, name="vcsum_col")
        act_warm = const.tile([1, 8], F16, name="act_warm")
        act_in = const.tile([1, 8], F16, name="act_in")

        nc.gpsimd.dma_start(ones[:], ones_d[:])
        nc.gpsimd.dma_start(bq_s[:], bq_d[:])
        nc.gpsimd.dma_start(bkv_s[:], bkv_d[:])
        nc.gpsimd.dma_start(mvts_s[:], mvts_d[:])
        # prefill the em (denominator) columns of vca8/vca16 from DRAM
        nc.gpsimd.dma_start(
            vca8[:].rearrange("p (j s c) -> p j s c", s=2, c=VSTR)[:, :, :,
                                                                  D:DP1],
            em8_d[:].rearrange("p (j s c) -> p j s c", s=2, c=1))
        nc.gpsimd.dma_start(
            vca16[:].rearrange("p (t c) -> p t c", c=DP1)[:, :, D:DP1],
            em16_d[:].rearrange("p (t c) -> p t c", c=1))
        # emsum goes into the denominator row of the vcsum column
        nc.sync.dma_start(vcsum_col[D:DP1, :], emsum_d[:])
        # memset-sourced input for the exp-table warm: no DMA dependency, so
        # the scalar engine reaches its hT dma_starts without stalling
        nc.vector.memset(act_in[:], 1.0)

        # ---- long-lived inputs (kv3 + mvt live until vc is done) ----
        iov = ctx.enter_context(tc.tile_pool(name="iov", bufs=1))
        kv3 = iov.tile([P, NT * 6 * D], F16, name="kv3")
        mvt_s = iov.tile([P, NT * KVL], F16, name="mvt_s")

        with tc.tile_pool(name="iok", bufs=1) as iok:
            mkt_s = iok.tile([P, NT * KVL], F16, name="mkt_s")

            # ---- Phase A ----
            with tc.tile_pool(name="ioa", bufs=1) as ioa:
                wq_s = ioa.tile([P, NT * H], F16, name="wq_s")
                hT_s = ioa.tile([P, NT * S], F16, name="hT_s")
                wkv_s = ioa.tile([P, NT * 6 * D], F16, name="wkv_s")

                # DMA queues: sync = wq, wkv then outputs; scalar = hT
                # only (keeps the ACT instruction stream clean for phase-C
                # exps); gpsimd = consts, mkt, then mvt.  wq/hT arrive in
                # kt-major layout, issued as 2-chunk groups: contiguous
                # 3 KiB partition lines, with per-group completion
                # semaphores so the kt-chunked Q proj starts on group 0.
                for g in range(NT // 2):
                    nc.sync.dma_start(wq_s[:, 2 * g * H:2 * (g + 1) * H],
                                      wq[:, 2 * g * H:2 * (g + 1) * H])
                    nc.scalar.dma_start(hT_s[:, 2 * g * S:2 * (g + 1) * S],
                                        hT[:, 2 * g * S:2 * (g + 1) * S])
                for kt in range(NT):
                    nc.sync.dma_start(wkv_s[:, kt * 6 * D:(kt + 1) * 6 * D],
                                      wkv[kt * P:(kt + 1) * P, :])
                # warm the exp table (ACT_TABLE_LOAD ~1.3us) during phase A
                nc.scalar.activation(act_warm[:], act_in[:], EXP, scale=64.0)
                for h in range(HK):
                    for st in range(NT):
                        nc.gpsimd.dma_start(
                            mkt_s[:, st * KVL + h * S: st * KVL + (h + 1) * S],
                            mkt[st * P:(st + 1) * P, h * S:(h + 1) * S])
                for h in range(HK):
                    for st in range(NT):
                        nc.gpsimd.dma_start(
                            mvt_s[:, st * KVL + h * S: st * KVL + (h + 1) * S],
                            mvt[st * P:(st + 1) * P, h * S:(h + 1) * S])

                # Q projection (pair-outer) + K/V projection.  qT is scaled
                # by 1/512 (= softmax 1/8 fused with the exp approximation's
                # 1/64) so phase C's DVE expm1 op needs no multiply stage.
                with tc.tile_pool(name="aps", bufs=2, space="PSUM") as aps:
                    for t in range(NPAIR):
                        q_ps = aps.tile([P, S], F32, name="q_ps", tag="q_ps")
                        for lo, hi in _bank_pieces(0, S):
                            for kt in range(NT):
                                nc.tensor.matmul(
                                    q_ps[:, lo:hi],
                                    wq_s[:, kt * H + t * P: kt * H + (t + 1) * P],
                                    hT_s[:, kt * S + lo: kt * S + hi],
                                    start=(kt == 0), stop=(kt == NT - 1))
                        if t % 2 == 0:
                            nc.vector.tensor_scalar(
                                qT[:, t * S:(t + 1) * S], q_ps[:],
                                bq_s[:, t:t + 1], QSC, op0=ADD, op1=MULT)
                        else:
                            nc.scalar.activation(
                                qT[:, t * S:(t + 1) * S], q_ps[:], IDT,
                                bias=bq_s[:, NPAIR + t:NPAIR + t + 1],
                                scale=QSC)

                    for st in range(NT):
                        kv_ps = aps.tile([P, 6 * D], F32, name="kv_ps",
                                         tag="kv_ps")
                        for kt in range(NT):
                            nc.tensor.matmul(
                                kv_ps[:],
                                hT_s[:, kt * S + st * P: kt * S + (st + 1) * P],
                                wkv_s[:, kt * 6 * D:(kt + 1) * 6 * D],
                                start=(kt == 0), stop=False)
                        nc.tensor.matmul(kv_ps[:], ones[:, 0:P], bkv_s[:],
                                         start=False, stop=True)
                        nc.vector.tensor_copy(
                            kv3[:, st * 6 * D:(st + 1) * 6 * D], kv_ps[:])

            # ---- Phase B: kc (kv-duplicated halves, concurrent col pairs)
            with tc.tile_pool(name="kcps", bufs=2, space="PSUM") as kcps:
                for h in range(HK):
                    kc_ps = kcps.tile([P, S], F32, name="kc_ps", tag="kc_ps")
                    for lo, hi in _bank_pieces(0, S):
                        for st in range(NT):
                            lhsT = kv3[:, st * 6 * D + h * D:
                                       st * 6 * D + (h + 1) * D]
                            rhs = mkt_s[:, st * KVL + h * S + lo:
                                        st * KVL + h * S + hi]
                            nc.tensor.matmul(
                                kc_ps[0:D, lo:hi], lhsT, rhs,
                                start=(st == 0), stop=(st == NT - 1))
                            nc.tensor.matmul(
                                kc_ps[D:P, lo:hi], lhsT, rhs,
                                start=(st == 0), stop=(st == NT - 1),
                                tile_position=(0, D))
                    nc.vector.tensor_copy(kcT[:, h * S:(h + 1) * S], kc_ps[:])

                # vcsum[d] = sum_{kv in DVE tiles} vc_em[kv, d]
                #          = sum_st mvts[st] @ v[st]  per DVE tile
                # (exact-in-fp16 correction term for the pr-1 shift trick)
                with tc.tile_pool(name="vsps", bufs=1, space="PSUM") as vsps:
                    vs_ps = vsps.tile([D, 1], F32, name="vs_ps")
                    n = 0
                    for i, t in enumerate(DVE_LIST):
                        h = t // NT
                        for st in range(NT):
                            nc.tensor.matmul(
                                vs_ps[:],
                                kv3[:, st * 6 * D + (HK + h) * D:
                                    st * 6 * D + (HK + h + 1) * D],
                                mvts_s[:, i * NT + st: i * NT + st + 1],
                                start=(n == 0), stop=(n == NDVE * NT - 1))
                            n += 1
                    nc.vector.tensor_copy(vcsum_col[0:D, :], vs_ps[:])

        # ---- Phase C ----
        # Act-path pr tiles: [p, head(2)*S] fp16 holding pr = exp(s);
        # DVE-path pair tiles: [p, slab(2), head(2)*S] fp8 with pr'=exp-1
        prp16 = ctx.enter_context(tc.tile_pool(name="prp16", bufs=22))
        prp8 = ctx.enter_context(tc.tile_pool(name="prp8", bufs=10))
        scps = ctx.enter_context(tc.tile_pool(name="scps", bufs=2,
                                              space="PSUM"))
        stg = ctx.enter_context(tc.tile_pool(name="stg", bufs=4))

        def emit_scores(p, t, dst):
            """Concurrent row-tiled pair: even head -> cols 0:768, odd
            head -> cols 768:1536 of a [128, 1536] psum tile; exp into
            the fp16 pr tile (Act tiles) or exp-1 into an fp8 pair-tile
            slab (DVE tiles)."""
            sc = scps.tile([P, 2 * S], F32, name="sc", tag="sc")
            kc_lo = kcT[0:D, t * P:(t + 1) * P]
            kc_hi = kcT[D:P, t * P:(t + 1) * P]
            for pa, pb in zip_longest(_bank_pieces(0, S),
                                      _bank_pieces(S, 2 * S)):
                if pa is not None:
                    lo, hi = pa
                    nc.tensor.matmul(sc[:, lo:hi], kc_lo,
                                     qT[0:D, p * S + lo: p * S + hi],
                                     start=True, stop=True)
                if pb is not None:
                    lob, hib = pb
                    nc.tensor.matmul(
                        sc[:, lob:hib], kc_hi,
                        qT[D:P, p * S + lob - S: p * S + hib - S],
                        start=True, stop=True)
            if t in ACT_SLOT:
                nc.scalar.activation(dst, sc[:], EXP, scale=64.0)
            else:
                nc.vector._custom_dve(expm1_op, out=dst, in0=sc[:])

        def emit_ctx16(a, pr16, half, ps, first, last):
            """fp16 ctx for Act-path tile slot a (unshifted pr)."""
            lhsT = vca16[:, a * DP1:(a + 1) * DP1]
            for lo, hi in _bank_pieces(0, S):
                nc.tensor.matmul(
                    ps[:, lo:hi], lhsT,
                    pr16[:, half * S + lo: half * S + hi],
                    start=first, stop=last)

        def emit_ctx8(j, pair, half, ps, first, last):
            """fp8 DoubleRow: DVE kv-tile pair j (K=256) in one pass."""
            lhsT = vca8[:, j * 2 * VSTR:(j + 1) * 2 * VSTR].rearrange(
                "p (s c) -> p s c", c=VSTR)[:, :, 0:DP1]
            rhs3 = pair[:].rearrange("p (s x) -> p s x", s=2)
            for lo, hi in _bank_pieces(0, S):
                nc.tensor.matmul(
                    ps[:, lo:hi], lhsT,
                    rhs3[:, :, half * S + lo: half * S + hi],
                    start=first, stop=last, perf_mode=DR)

        def flush_ctx(ps, head):
            # add the shift-trick correction (vcsum / emsum) while copying
            # psum -> sbuf, split across the two exp engines
            st_t = stg.tile([DP1, S], F32, name="st_t", tag="st")
            nc.vector.tensor_scalar_add(st_t[:, 0:SH], ps[:, 0:SH],
                                        vcsum_col[:, 0:1])
            nc.scalar.activation(st_t[:, SH:S], ps[:, SH:S], IDT,
                                 bias=vcsum_col[:, 0:1])
            nc.sync.dma_start(out_d[head], st_t[:])

        def emit_vc(t):
            h = t // NT
            vc_ps = vcps.tile([P, D], F32, name="vc_ps", tag="vc")
            for st in range(NT):
                nc.tensor.matmul(
                    vc_ps[:],
                    mvt_s[:, st * KVL + t * P: st * KVL + (t + 1) * P],
                    kv3[:, st * 6 * D + (HK + h) * D:
                        st * 6 * D + (HK + h + 1) * D],
                    start=(st == 0), stop=(st == NT - 1))
            if t in ACT_SLOT:
                a = ACT_SLOT[t]
                nc.vector.tensor_copy(
                    vca16[:, a * DP1: a * DP1 + D], vc_ps[:])
            else:
                i = DVE_SLOT[t]
                nc.vector.tensor_copy(
                    vca8[:, (i // 2) * 2 * VSTR + (i % 2) * VSTR:
                         (i // 2) * 2 * VSTR + (i % 2) * VSTR + D],
                    vc_ps[:])

        def alloc_pr(t, store):
            """Allocate/locate the exp destination for kv tile t."""
            if t in ACT_SLOT:
                pr16 = prp16.tile([P, 2 * S], F16, name="pr16", tag="pr16")
                store["act"].append(pr16)
                return pr16[:]
            i = DVE_SLOT[t]
            if i % 2 == 0:
                store["dve"].append(
                    prp8.tile([P, 4 * S], F8, name="pr8", tag="pr8"))
            pair = store["dve"][-1]
            return pair[:, (i % 2) * 2 * S:(i % 2 + 1) * 2 * S]

        # ctx work items per head-half: 10 fp16 groups + 4 fp8-DR groups
        NITEM = NACT + NPR8

        def emit_ctx_item(k, prev, half, ps):
            if k < NACT:
                emit_ctx16(k, prev["act"][k], half, ps,
                           first=(k == 0), last=(k == NITEM - 1))
            else:
                emit_ctx8(k - NACT, prev["dve"][k - NACT], half, ps,
                          first=(k == 0), last=(k == NITEM - 1))

        # sweep 0: scores/exp for pair 0 with the vc matmuls interleaved
        # (PE filler under the engine-paced exp window)
        prev = {"act": [], "dve": []}
        with tc.tile_pool(name="vcps", bufs=2, space="PSUM") as vcps:
            for t in range(NKT):
                emit_scores(0, t, alloc_pr(t, prev))
                emit_vc(t)

        # sweeps 1..6: scores/exp for pair p + deferred ctx chains for
        # pair p-1 (A then B) sharing one psum accumulator.  Scores are
        # front-loaded (done ~85% through the sweep) so the exp tail
        # drains while the PE runs chain B's pure-ctx tail -- the next
        # sweep's first scores then find a free scores-psum buffer.
        with tc.tile_pool(name="ctxps", bufs=1, space="PSUM") as ctxps:
            for p in range(1, NPAIR + 1):
                cur = {"act": [], "dve": []}
                events = []
                if p < NPAIR:
                    # interleave: sc0 sc1 [A0] sc2 [A1] ... sc13 [A12 A13]
                    # FA sc14 [B0] sc15 [B1] sc16 [B2] sc17 [B3..B13] FB
                    events.append(("sc", 0))
                    for t in range(1, 14):
                        events.append(("sc", t))
                        events.append(("A", t - 1))
                    events += [("A", 13), ("FA", None)]
                    for t in range(14, NKT):
                        events.append(("sc", t))
                        events.append(("B", t - 14))
                    events += [("B", j) for j in range(4, NITEM)]
                    events.append(("FB", None))
                else:
                    events = ([("A", k) for k in range(NITEM)]
                              + [("FA", None)]
                              + [("B", k) for k in range(NITEM)]
                              + [("FB", None)])
                ps = None
                for kind, v in events:
                    if kind == "sc":
                        emit_scores(p, v, alloc_pr(v, cur))
                    elif kind == "A":
                        if v == 0:
                            ps = ctxps.tile([DP1, S], F32, name="ctx",
                                            tag="ctx")
                        emit_ctx_item(v, prev, 0, ps)
                    elif kind == "B":
                        if v == 0:
                            ps = ctxps.tile([DP1, S], F32, name="ctx",
                                            tag="ctx")
                        emit_ctx_item(v, prev, 1, ps)
                    elif kind == "FA":
                        flush_ctx(ps, 2 * (p - 1))
                    else:
                        flush_ctx(ps, 2 * (p - 1) + 1)
                prev = cur

    nc.compile()
    return nc


def _get_program():
    global _PROGRAM
    if _PROGRAM is None:
        _PROGRAM = _build_program()
    return _PROGRAM


def kernel(hidden_states, attention_mask, Wq, bq, Wk, bk, Wv, bv, gate,
           mem_keys, mem_values):
    import ml_dtypes
    from concourse.bass_utils import run_bass_kernel_spmd

    global LAST_RESULTS

    f32, f16 = np.float32, np.float16
    f8 = ml_dtypes.float8_e4m3
    hidden_states = np.asarray(hidden_states, f32)
    attention_mask = np.asarray(attention_mask, f32)
    Wq = np.asarray(Wq, f32)
    bq = np.asarray(bq, f32)
    Wk = np.asarray(Wk, f32)
    bk = np.asarray(bk, f32)
    Wv = np.asarray(Wv, f32)
    bv = np.asarray(bv, f32)
    gate = np.asarray(gate, f32)
    mem_keys = np.asarray(mem_keys, f32)
    mem_values = np.asarray(mem_values, f32)

    # kt-major device layouts: x_kt[p, kt*C + c] = x[kt*128 + p, c]
    hT16 = [np.ascontiguousarray(
                hidden_states[b].T.reshape(NT, P, S).transpose(1, 0, 2)
                .reshape(P, NT * S)).astype(f16)
            for b in range(B)]
    wq16 = np.ascontiguousarray(
        Wq.reshape(NT, P, H).transpose(1, 0, 2).reshape(P, NT * H)
    ).astype(f16)
    bq_pair = np.ascontiguousarray(
        bq.reshape(NPAIR, 2, D).transpose(1, 2, 0).reshape(P, NPAIR))
    bq_dev = np.concatenate([bq_pair, bq_pair * QSC], axis=1)
    em_full = np.exp(attention_mask.reshape(B, NH * S)).astype(f32)
    ones_dev = np.ones((1, S), f16)

    in_maps = []
    for c in range(NCORES):
        b, quart = c // NQUART, c % NQUART
        heads = [HK * quart + j for j in range(HK)]
        wkv_c = np.concatenate(
            [Wk[:, h * D:(h + 1) * D] for h in heads]
            + [Wv[:, h * D:(h + 1) * D] for h in heads], axis=1)
        bkv_c = np.concatenate(
            [bk[h * D:(h + 1) * D] for h in heads]
            + [bv[h * D:(h + 1) * D] for h in heads])[None, :]
        # mkt[s, h_local*768+kv] = mem_keys[heads[h_local], kv, s]
        mkt_c = mem_keys[heads].transpose(2, 0, 1).reshape(S, KVL)
        em_c = em_full[b, quart * KVL:(quart + 1) * KVL]
        mvt_c = (mem_values[heads].transpose(2, 0, 1).reshape(S, KVL)
                 * em_c[None, :])
        em_tiles = em_c.reshape(NKT, P)                      # [t, p]
        em16_dev = np.ascontiguousarray(em_tiles[list(ACT_LIST)].T)
        em8_dev = np.ascontiguousarray(em_tiles[list(DVE_LIST)].T)
        # per-DVE-tile row sums of mvt (em-weighted), kt-chunked:
        # mvts_dev[p, i*NT+st] = sum_{kv in DVE tile i} mvt[st*128+p, kv]
        msum = mvt_c.reshape(S, NKT, P).sum(axis=2)          # [S, NKT]
        msum = msum[:, list(DVE_LIST)]                       # [S, NDVE]
        mvts_dev = np.ascontiguousarray(
            msum.reshape(NT, P, NDVE).transpose(1, 2, 0).reshape(P,
                                                                 NDVE * NT))
        emsum_dve = em_tiles[list(DVE_LIST)].sum()
        in_maps.append({
            "hT": hT16[b],
            "wq": wq16,
            "bq_d": bq_dev,
            "wkv": np.ascontiguousarray(wkv_c).astype(f16),
            "bkv_d": np.ascontiguousarray(bkv_c).astype(f16),
            "mkt": np.ascontiguousarray(mkt_c).astype(f16),
            "mvt": np.ascontiguousarray(mvt_c).astype(f16),
            "em8_d": em8_dev.astype(f8),
            "em16_d": em16_dev.astype(f16),
            "mvts_d": mvts_dev.astype(f16),
            "emsum_d": np.array([[emsum_dve]], f32),
            "ones_d": ones_dev,
        })

    nc = _get_program()
    res = run_bass_kernel_spmd(nc, in_maps, core_ids=list(range(NCORES)),
                               trace=TRACE)
    LAST_RESULTS = res

    out = np.empty((B, S, NH, D), f32)
    for b in range(B):
        parts = res.results[b * NQUART]["out_d"].astype(f32).copy()
        for c in range(b * NQUART + 1, (b + 1) * NQUART):
            parts += res.results[c]["out_d"]
        num = parts[:, :D, :]                     # [12, 64, 768]
        den = parts[:, D, :]                      # [12, 768]
        ctxT = num / den[:, None, :]
        out[b] = ctxT.transpose(2, 0, 1)          # [768, 12, 64]
    g = (1.0 / (1.0 + np.exp(-gate))).reshape(1, 1, NH, 1)
    return (g * out).astype(f32)


# revision 26
# speedup vs baseline: 1.0220x; 1.0106x over previous
"""Trainium2 Bass kernel for BertInfiniSelfAttention — v3 design.

Sharding (8 cores): core c = (batch b = c//4, kv-quarter q = c%4).
Each core owns batch b and kv heads {3q, 3q+1, 3q+2} (KVL = 2304 of the
9216 concatenated kv positions), computes the full Q projection for its
batch, the K/V projections + memory matmuls for its 3 kv heads, then
flash-style partial attention for all 12 q heads against its local KV.
Host sums partial (numerator, denominator) over the 4 kv-quarters per
batch and divides.

v3: the ctx (probs @ V) matmuls run in fp8e4 DoubleRow mode — two kv
tiles (K=256) per pass at 1 column/cycle, 2x the fp16 rate.  Direct fp8
quantization of probs/values would exceed the error budget (softmax
averages 9216 near-uniform weights, so 3.6% rms fp8 noise survives into
the output), so the kernel uses a shift trick:

    ctx = sum_kv pr*vc = (sum_kv vc) + sum_kv (pr-1)*vc

The first term (vcsum) is computed exactly in fp16 via host-side row
sums of mvt (vcsum = mvtsum @ v, 18 N=1 matmuls) and added at flush
time; only the small residual pr' = exp(s)-1 (|pr'| ~ 0.17) and vc are
fp8, so quantization noise lands on a 6x smaller term.  The denominator
row works the same way: row 64 accumulates em*pr', and emsum (host
constant) is added at flush.

pr' production: qT is pre-scaled by 1/512 so scores psum holds s/64.
DVE tiles use a custom op (1+x)^64 - 1 (8 ALU stages); Act tiles
compute exp(64x) into fp16 staging and the otherwise-idle GpSimd engine
applies -1 with an fp8 cast.

Startup: wq/hT arrive in kt-major layout and are DMAed in 2-chunk
groups with per-group semaphores so the Q projection starts on group 0
(~12us) instead of after the full 2.4 MB (~20us).
"""

import numpy as np

B, S, H, NH, D = 2, 768, 768, 12, 64
P = 128
NCORES = 8
NQUART = 4              # kv quarters
HK = 3                  # kv heads per core
KVL = HK * S            # 2304 local kv
NKT = KVL // P          # 18 kv tiles
ACT_LIST = (0, 1, 2, 4, 6, 8, 10, 12, 14, 15)   # fp16 path (Act exp)
DVE_LIST = (3, 5, 7, 9, 11, 13, 16, 17)          # fp8 path (DVE expm1)
NACT = len(ACT_LIST)    # 10
NDVE = len(DVE_LIST)    # 8
NPR8 = NDVE // 2        # 4 fp8 DoubleRow kv-tile pairs
NT = S // P             # 6 s/H tiles
DP1 = D + 1
VSTR = 80               # vca8 slab stride (DP1 padded to a mult of 16)
NPAIR = NH // 2         # 6 q-head pairs
SH = S // 2             # 384, flush s-half
QSC = 1.0 / 512.0       # qT pre-scale: 1/sqrt(D)/64 folded into qT

ACT_SLOT = {t: i for i, t in enumerate(ACT_LIST)}
DVE_SLOT = {t: i for i, t in enumerate(DVE_LIST)}

_PROGRAM = None
TRACE = False
LAST_RESULTS = None


def _bank_pieces(lo, hi):
    """Split [lo,hi) free-dim range at 512-fp32 PSUM bank boundaries."""
    out = []
    while lo < hi:
        nxt = min(hi, (lo // 512 + 1) * 512)
        out.append((lo, nxt))
        lo = nxt
    return out


def _expm1_ref(in0, in1, s0, s1, imm2):
    t = (in0 + 1.0).astype(np.float32)
    for _ in range(6):
        t = t * t
    return t - 1.0


def _make_expm1_op():
    from concourse import dve_ops as DO
    from concourse.dve_spec import Spec, Src0, One, sq

    for o in DO.OPS:
        if o.name == "EXPM1_SQ64_ANT":
            return o
    body = sq(sq(sq(sq(sq(sq(Src0 + One)))))) - One
    op = DO.DveOp(
        "EXPM1_SQ64_ANT",
        Spec(body=body, reference=_expm1_ref),
        subdim=False,
        uops_sha={"v3": "0d629377a67c4031", "v4": "a8d0e57c9f1ce618"},
    )
    DO.OPS.append(op)
    DO._SUB_OPCODE_FOR_NAME[op.name] = DO._CUSTOM_DVE_ROW_BASE + len(DO.OPS) - 1
    return op


def _build_program():
    from contextlib import ExitStack
    from itertools import zip_longest

    import concourse.bacc as bacc
    import concourse.mybir as mybir
    import concourse.tile as tile

    expm1_op = _make_expm1_op()

    F32 = mybir.dt.float32
    F16 = mybir.dt.float16
    F8 = mybir.dt.float8e4
    EXP = mybir.ActivationFunctionType.Exp
    IDT = mybir.ActivationFunctionType.Identity
    DR = mybir.MatmulPerfMode.DoubleRow
    ADD = mybir.AluOpType.add
    MULT = mybir.AluOpType.mult

    nc = bacc.Bacc("TRN2", target_bir_lowering=False, debug=False,
                   num_devices=NCORES)

    # hT / wq come in kt-major layout ([p, kt, c], pre-shuffled on the host)
    # so chunk-group DMAs are contiguous with 3 KiB per partition line
    hT = nc.declare_dram_parameter("hT", [P, NT * S], F16, isOutput=False)
    wq = nc.declare_dram_parameter("wq", [P, NT * H], F16, isOutput=False)
    # bq_d cols 0:NPAIR = bq (pair layout); NPAIR:2*NPAIR = bq/512
    bq_d = nc.declare_dram_parameter("bq_d", [P, 2 * NPAIR], F32,
                                     isOutput=False)
    wkv = nc.declare_dram_parameter("wkv", [H, 6 * D], F16, isOutput=False)
    bkv_d = nc.declare_dram_parameter("bkv_d", [1, 6 * D], F16, isOutput=False)
    mkt = nc.declare_dram_parameter("mkt", [S, KVL], F16, isOutput=False)
    mvt = nc.declare_dram_parameter("mvt", [S, KVL], F16, isOutput=False)
    em8_d = nc.declare_dram_parameter("em8_d", [P, NDVE], F8, isOutput=False)
    em16_d = nc.declare_dram_parameter("em16_d", [P, NACT], F16,
                                       isOutput=False)
    # mvts_d[p, i*NT+st] = sum_{kv in DVE tile i} mvt[st*128+p, kv]  (fp16)
    mvts_d = nc.declare_dram_parameter("mvts_d", [P, NDVE * NT], F16,
                                       isOutput=False)
    emsum_d = nc.declare_dram_parameter("emsum_d", [1, 1], F32,
                                        isOutput=False)
    ones_d = nc.declare_dram_parameter("ones_d", [1, S], F16, isOutput=False)
    out_d = nc.declare_dram_parameter("out_d", [NH, DP1, S], F32, isOutput=True)

    with tile.TileContext(nc) as tc, ExitStack() as ctx:
        const = ctx.enter_context(tc.tile_pool(name="const", bufs=1))

        qT = const.tile([P, NPAIR * S], F16, name="qT")
        kcT = const.tile([P, KVL], F16, name="kcT")
        # vca16: fp16 vc+em for the Act-path tiles (ACT_LIST order)
        vca16 = const.tile([P, NACT * DP1], F16, name="vca16")
        # vca8[p, j, s, c]: DVE kv-tile pair j, slab s, c = 64 vc cols +
        # em col, padded to VSTR=80 so the DoubleRow weights AP slab step
        # is a multiple of 16 (s3_lw dual-fp8 ISA restriction)
        vca8 = const.tile([P, NPR8 * 2 * VSTR], F8, name="vca8")
        ones = const.tile([1, S], F16, name="ones")
        bq_s = const.tile([P, 2 * NPAIR], F32, name="bq_s")
        bkv_s = const.tile([1, 6 * D], F16, name="bkv_s")
        mvts_s = const.tile([P, NDVE * NT], F16, name="mvts_s")
        vcsum_col = const.tile([DP1, 1], F32, name="vcsum_col")
        act_warm = const.tile([1, 8], F16, name="act_warm")
        act_in = const.tile([1, 8], F16, name="act_in")

        nc.gpsimd.dma_start(ones[:], ones_d[:])
        nc.gpsimd.dma_start(bq_s[:], bq_d[:])
        nc.gpsimd.dma_start(bkv_s[:], bkv_d[:])
        nc.gpsimd.dma_start(mvts_s[:], mvts_d[:])
        # prefill the em (denominator) columns of vca8/vca16 from DRAM
        nc.gpsimd.dma_start(
            vca8[:].rearrange("p (j s c) -> p j s c", s=2, c=VSTR)[:, :, :,
                                                                  D:DP1],
            em8_d[:].rearrange("p (j s c) -> p j s c", s=2, c=1))
        nc.gpsimd.dma_start(
            vca16[:].rearrange("p (t c) -> p t c", c=DP1)[:, :, D:DP1],
            em16_d[:].rearrange("p (t c) -> p t c", c=1))
        # emsum goes into the denominator row of the vcsum column
        nc.sync.dma_start(vcsum_col[D:DP1, :], emsum_d[:])
        # memset-sourced input for the exp-table warm: no DMA dependency, so
        # the scalar engine reaches its hT dma_starts without stalling
        nc.vector.memset(act_in[:], 1.0)

        # ---- long-lived inputs (kv3 + mvt live until vc is done) ----
        iov = ctx.enter_context(tc.tile_pool(name="iov", bufs=1))
        kv3 = iov.tile([P, NT * 6 * D], F16, name="kv3")
        mvt_s = iov.tile([P, NT * KVL], F16, name="mvt_s")

        with tc.tile_pool(name="iok", bufs=1) as iok:
            mkt_s = iok.tile([P, NT * KVL], F16, name="mkt_s")

            # ---- Phase A ----
            with tc.tile_pool(name="ioa", bufs=1) as ioa:
                wq_s = ioa.tile([P, NT * H], F16, name="wq_s")
                hT_s = ioa.tile([P, NT * S], F16, name="hT_s")
                wkv_s = ioa.tile([P, NT * 6 * D], F16, name="wkv_s")

                # DMA queues: sync = wq, wkv then outputs; scalar = hT
                # only (keeps the ACT instruction stream clean for phase-C
                # exps); gpsimd = consts, mkt, then mvt.  wq/hT arrive in
                # kt-major layout, issued as 2-chunk groups: contiguous
                # 3 KiB partition lines, with per-group completion
                # semaphores so the kt-chunked Q proj starts on group 0.
                for g in range(NT // 2):
                    nc.sync.dma_start(wq_s[:, 2 * g * H:2 * (g + 1) * H],
                                      wq[:, 2 * g * H:2 * (g + 1) * H])
                    nc.scalar.dma_start(hT_s[:, 2 * g * S:2 * (g + 1) * S],
                                        hT[:, 2 * g * S:2 * (g + 1) * S])
                for kt in range(NT):
                    nc.sync.dma_start(wkv_s[:, kt * 6 * D:(kt + 1) * 6 * D],
                                      wkv[kt * P:(kt + 1) * P, :])
                # warm the exp table (ACT_TABLE_LOAD ~1.3us) during phase A
                nc.scalar.activation(act_warm[:], act_in[:], EXP, scale=64.0)
                for h in range(HK):
                    for st in range(NT):
                        nc.gpsimd.dma_start(
                            mkt_s[:, st * KVL + h * S: st * KVL + (h + 1) * S],
                            mkt[st * P:(st + 1) * P, h * S:(h + 1) * S])
                for h in range(HK):
                    for st in range(NT):
                        nc.gpsimd.dma_start(
                            mvt_s[:, st * KVL + h * S: st * KVL + (h + 1) * S],
                            mvt[st * P:(st + 1) * P, h * S:(h + 1) * S])

                # Q projection (pair-outer) + K/V projection.  qT is scaled
                # by 1/512 (= softmax 1/8 fused with the exp approximation's
                # 1/64) so phase C's DVE expm1 op needs no multiply stage.
                with tc.tile_pool(name="aps", bufs=2, space="PSUM") as aps:
                    for t in range(NPAIR):
                        q_ps = aps.tile([P, S], F32, name="q_ps", tag="q_ps")
                        for lo, hi in _bank_pieces(0, S):
                            for kt in range(NT):
                                nc.tensor.matmul(
                                    q_ps[:, lo:hi],
                                    wq_s[:, kt * H + t * P: kt * H + (t + 1) * P],
                                    hT_s[:, kt * S + lo: kt * S + hi],
                                    start=(kt == 0), stop=(kt == NT - 1))
                        if t % 2 == 0:
                            nc.vector.tensor_scalar(
                                qT[:, t * S:(t + 1) * S], q_ps[:],
                                bq_s[:, t:t + 1], QSC, op0=ADD, op1=MULT)
                        else:
                            nc.scalar.activation(
                                qT[:, t * S:(t + 1) * S], q_ps[:], IDT,
                                bias=bq_s[:, NPAIR + t:NPAIR + t + 1],
                                scale=QSC)

                    for st in range(NT):
                        kv_ps = aps.tile([P, 6 * D], F32, name="kv_ps",
                                         tag="kv_ps")
                        for kt in range(NT):
                            nc.tensor.matmul(
                                kv_ps[:],
                                hT_s[:, kt * S + st * P: kt * S + (st + 1) * P],
                                wkv_s[:, kt * 6 * D:(kt + 1) * 6 * D],
                                start=(kt == 0), stop=False)
                        nc.tensor.matmul(kv_ps[:], ones[:, 0:P], bkv_s[:],
                                         start=False, stop=True)
                        nc.vector.tensor_copy(
                            kv3[:, st * 6 * D:(st + 1) * 6 * D], kv_ps[:])

            # ---- Phase B: kc (kv-duplicated halves, concurrent col pairs)
            with tc.tile_pool(name="kcps", bufs=2, space="PSUM") as kcps:
                for h in range(HK):
                    kc_ps = kcps.tile([P, S], F32, name="kc_ps", tag="kc_ps")
                    for lo, hi in _bank_pieces(0, S):
                        for st in range(NT):
                            lhsT = kv3[:, st * 6 * D + h * D:
                                       st * 6 * D + (h + 1) * D]
                            rhs = mkt_s[:, st * KVL + h * S + lo:
                                        st * KVL + h * S + hi]
                            nc.tensor.matmul(
                                kc_ps[0:D, lo:hi], lhsT, rhs,
                                start=(st == 0), stop=(st == NT - 1))
                            nc.tensor.matmul(
                                kc_ps[D:P, lo:hi], lhsT, rhs,
                                start=(st == 0), stop=(st == NT - 1),
                                tile_position=(0, D))
                    nc.vector.tensor_copy(kcT[:, h * S:(h + 1) * S], kc_ps[:])

                # vcsum[d] = sum_{kv in DVE tiles} vc_em[kv, d]
                #          = sum_st mvts[st] @ v[st]  per DVE tile
                # (exact-in-fp16 correction term for the pr-1 shift trick)
                with tc.tile_pool(name="vsps", bufs=1, space="PSUM") as vsps:
                    vs_ps = vsps.tile([D, 1], F32, name="vs_ps")
                    n = 0
                    for i, t in enumerate(DVE_LIST):
                        h = t // NT
                        for st in range(NT):
                            nc.tensor.matmul(
                                vs_ps[:],
                                kv3[:, st * 6 * D + (HK + h) * D:
                                    st * 6 * D + (HK + h + 1) * D],
                                mvts_s[:, i * NT + st: i * NT + st + 1],
                                start=(n == 0), stop=(n == NDVE * NT - 1))
                            n += 1
                    nc.vector.tensor_copy(vcsum_col[0:D, :], vs_ps[:])

        # ---- Phase C ----
        # Act-path pr tiles: [p, head(2)*S] fp16 holding pr = exp(s);
        # DVE-path pair tiles: [p, slab(2), head(2)*S] fp8 with pr'=exp-1
        prp16 = ctx.enter_context(tc.tile_pool(name="prp16", bufs=22))
        prp8 = ctx.enter_context(tc.tile_pool(name="prp8", bufs=10))
        scps = ctx.enter_context(tc.tile_pool(name="scps", bufs=2,
                                              space="PSUM"))
        stg = ctx.enter_context(tc.tile_pool(name="stg", bufs=4))

        def emit_scores(p, t, dst):
            """Concurrent row-tiled pair: even head -> cols 0:768, odd
            head -> cols 768:1536 of a [128, 1536] psum tile; exp into
            the fp16 pr tile (Act tiles) or exp-1 into an fp8 pair-tile
            slab (DVE tiles)."""
            sc = scps.tile([P, 2 * S], F32, name="sc", tag="sc")
            kc_lo = kcT[0:D, t * P:(t + 1) * P]
            kc_hi = kcT[D:P, t * P:(t + 1) * P]
            for pa, pb in zip_longest(_bank_pieces(0, S),
                                      _bank_pieces(S, 2 * S)):
                if pa is not None:
                    lo, hi = pa
                    nc.tensor.matmul(sc[:, lo:hi], kc_lo,
                                     qT[0:D, p * S + lo: p * S + hi],
                                     start=True, stop=True)
                if pb is not None:
                    lob, hib = pb
                    nc.tensor.matmul(
                        sc[:, lob:hib], kc_hi,
                        qT[D:P, p * S + lob - S: p * S + hib - S],
                        start=True, stop=True)
            if t in ACT_SLOT:
                nc.scalar.activation(dst, sc[:], EXP, scale=64.0)
            else:
                nc.vector._custom_dve(expm1_op, out=dst, in0=sc[:])

        def emit_ctx16(a, pr16, half, ps, first, last):
            """fp16 ctx for Act-path tile slot a (unshifted pr)."""
            lhsT = vca16[:, a * DP1:(a + 1) * DP1]
            for lo, hi in _bank_pieces(0, S):
                nc.tensor.matmul(
                    ps[:, lo:hi], lhsT,
                    pr16[:, half * S + lo: half * S + hi],
                    start=first, stop=last)

        def emit_ctx8(j, pair, half, ps, first, last):
            """fp8 DoubleRow: DVE kv-tile pair j (K=256) in one pass."""
            lhsT = vca8[:, j * 2 * VSTR:(j + 1) * 2 * VSTR].rearrange(
                "p (s c) -> p s c", c=VSTR)[:, :, 0:DP1]
            rhs3 = pair[:].rearrange("p (s x) -> p s x", s=2)
            for lo, hi in _bank_pieces(0, S):
                nc.tensor.matmul(
                    ps[:, lo:hi], lhsT,
                    rhs3[:, :, half * S + lo: half * S + hi],
                    start=first, stop=last, perf_mode=DR)

        def flush_ctx(ps, head):
            # add the shift-trick correction (vcsum / emsum) while copying
            # psum -> sbuf, split across the two exp engines
            st_t = stg.tile([DP1, S], F32, name="st_t", tag="st")
            nc.vector.tensor_scalar_add(st_t[:, 0:SH], ps[:, 0:SH],
                                        vcsum_col[:, 0:1])
            nc.scalar.activation(st_t[:, SH:S], ps[:, SH:S], IDT,
                                 bias=vcsum_col[:, 0:1])
            nc.sync.dma_start(out_d[head], st_t[:])

        def emit_vc(t):
            h = t // NT
            vc_ps = vcps.tile([P, D], F32, name="vc_ps", tag="vc")
            for st in range(NT):
                nc.tensor.matmul(
                    vc_ps[:],
                    mvt_s[:, st * KVL + t * P: st * KVL + (t + 1) * P],
                    kv3[:, st * 6 * D + (HK + h) * D:
                        st * 6 * D + (HK + h + 1) * D],
                    start=(st == 0), stop=(st == NT - 1))
            if t in ACT_SLOT:
                a = ACT_SLOT[t]
                nc.vector.tensor_copy(
                    vca16[:, a * DP1: a * DP1 + D], vc_ps[:])
            else:
                i = DVE_SLOT[t]
                nc.vector.tensor_copy(
                    vca8[:, (i // 2) * 2 * VSTR + (i % 2) * VSTR:
                         (i // 2) * 2 * VSTR + (i % 2) * VSTR + D],
                    vc_ps[:])

        def alloc_pr(t, store):
            """Allocate/locate the exp destination for kv tile t."""
            if t in ACT_SLOT:
                pr16 = prp16.tile([P, 2 * S], F16, name="pr16", tag="pr16")
                store["act"].append(pr16)
                return pr16[:]
            i = DVE_SLOT[t]
            if i % 2 == 0:
                store["dve"].append(
                    prp8.tile([P, 4 * S], F8, name="pr8", tag="pr8"))
            pair = store["dve"][-1]
            return pair[:, (i % 2) * 2 * S:(i % 2 + 1) * 2 * S]

        # ctx work items per head-half: 10 fp16 groups + 4 fp8-DR groups
        NITEM = NACT + NPR8

        def emit_ctx_item(k, prev, half, ps):
            if k < NACT:
                emit_ctx16(k, prev["act"][k], half, ps,
                           first=(k == 0), last=(k == NITEM - 1))
            else:
                emit_ctx8(k - NACT, prev["dve"][k - NACT], half, ps,
                          first=(k == 0), last=(k == NITEM - 1))

        # sweep 0: scores/exp for pair 0 with the vc matmuls interleaved
        # (PE filler under the engine-paced exp window)
        prev = {"act": [], "dve": []}
        with tc.tile_pool(name="vcps", bufs=2, space="PSUM") as vcps:
            for t in range(NKT):
                emit_scores(0, t, alloc_pr(t, prev))
                emit_vc(t)

        # sweeps 1..6: scores/exp for pair p + deferred ctx chains for
        # pair p-1 (A then B) sharing one psum accumulator.  Scores are
        # front-loaded (done ~85% through the sweep) so the exp tail
        # drains while the PE runs chain B's pure-ctx tail -- the next
        # sweep's first scores then find a free scores-psum buffer.
        with tc.tile_pool(name="ctxps", bufs=1, space="PSUM") as ctxps:
            for p in range(1, NPAIR + 1):
                cur = {"act": [], "dve": []}
                events = []
                if p < NPAIR:
                    # sc0 sc1 sc2 [A0] sc3 [A1] ... sc15 [A13] FA sc16
                    # sc17, then the whole B chain as a pure-ctx tail that
                    # keeps the PE busy while the exp queues drain
                    events += [("sc", 0), ("sc", 1)]
                    for t in range(2, 16):
                        events.append(("sc", t))
                        events.append(("A", t - 2))
                    events += [("FA", None), ("sc", 16), ("sc", 17)]
                    events += [("B", j) for j in range(NITEM)]
                    events.append(("FB", None))
                else:
                    events = ([("A", k) for k in range(NITEM)]
                              + [("FA", None)]
                              + [("B", k) for k in range(NITEM)]
                              + [("FB", None)])
                ps = None
                for kind, v in events:
                    if kind == "sc":
                        emit_scores(p, v, alloc_pr(v, cur))
                    elif kind == "A":
                        if v == 0:
                            ps = ctxps.tile([DP1, S], F32, name="ctx",
                                            tag="ctx")
                        emit_ctx_item(v, prev, 0, ps)
                    elif kind == "B":
                        if v == 0:
                            ps = ctxps.tile([DP1, S], F32, name="ctx",
                                            tag="ctx")
                        emit_ctx_item(v, prev, 1, ps)
                    elif kind == "FA":
                        flush_ctx(ps, 2 * (p - 1))
                    else:
                        flush_ctx(ps, 2 * (p - 1) + 1)
                prev = cur

    nc.compile()
    return nc


def _get_program():
    global _PROGRAM
    if _PROGRAM is None:
        _PROGRAM = _build_program()
    return _PROGRAM


def kernel(hidden_states, attention_mask, Wq, bq, Wk, bk, Wv, bv, gate,
           mem_keys, mem_values):
    import ml_dtypes
    from concourse.bass_utils import run_bass_kernel_spmd

    global LAST_RESULTS

    f32, f16 = np.float32, np.float16
    f8 = ml_dtypes.float8_e4m3
    hidden_states = np.asarray(hidden_states, f32)
    attention_mask = np.asarray(attention_mask, f32)
    Wq = np.asarray(Wq, f32)
    bq = np.asarray(bq, f32)
    Wk = np.asarray(Wk, f32)
    bk = np.asarray(bk, f32)
    Wv = np.asarray(Wv, f32)
    bv = np.asarray(bv, f32)
    gate = np.asarray(gate, f32)
    mem_keys = np.asarray(mem_keys, f32)
    mem_values = np.asarray(mem_values, f32)

    # kt-major device layouts: x_kt[p, kt*C + c] = x[kt*128 + p, c]
    hT16 = [np.ascontiguousarray(
                hidden_states[b].T.reshape(NT, P, S).transpose(1, 0, 2)
                .reshape(P, NT * S)).astype(f16)
            for b in range(B)]
    wq16 = np.ascontiguousarray(
        Wq.reshape(NT, P, H).transpose(1, 0, 2).reshape(P, NT * H)
    ).astype(f16)
    bq_pair = np.ascontiguousarray(
        bq.reshape(NPAIR, 2, D).transpose(1, 2, 0).reshape(P, NPAIR))
    bq_dev = np.concatenate([bq_pair, bq_pair * QSC], axis=1)
    em_full = np.exp(attention_mask.reshape(B, NH * S)).astype(f32)
    ones_dev = np.ones((1, S), f16)

    in_maps = []
    for c in range(NCORES):
        b, quart = c // NQUART, c % NQUART
        heads = [HK * quart + j for j in range(HK)]
        wkv_c = np.concatenate(
            [Wk[:, h * D:(h + 1) * D] for h in heads]
            + [Wv[:, h * D:(h + 1) * D] for h in heads], axis=1)
        bkv_c = np.concatenate(
            [bk[h * D:(h + 1) * D] for h in heads]
            + [bv[h * D:(h + 1) * D] for h in heads])[None, :]
        # mkt[s, h_local*768+kv] = mem_keys[heads[h_local], kv, s]
        mkt_c = mem_keys[heads].transpose(2, 0, 1).reshape(S, KVL)
        em_c = em_full[b, quart * KVL:(quart + 1) * KVL]
        mvt_c = (mem_values[heads].transpose(2, 0, 1).reshape(S, KVL)
                 * em_c[None, :])
        em_tiles = em_c.reshape(NKT, P)                      # [t, p]
        em16_dev = np.ascontiguousarray(em_tiles[list(ACT_LIST)].T)
        em8_dev = np.ascontiguousarray(em_tiles[list(DVE_LIST)].T)
        # per-DVE-tile row sums of mvt (em-weighted), kt-chunked:
        # mvts_dev[p, i*NT+st] = sum_{kv in DVE tile i} mvt[st*128+p, kv]
        msum = mvt_c.reshape(S, NKT, P).sum(axis=2)          # [S, NKT]
        msum = msum[:, list(DVE_LIST)]                       # [S, NDVE]
        mvts_dev = np.ascontiguousarray(
            msum.reshape(NT, P, NDVE).transpose(1, 2, 0).reshape(P,
                                                                 NDVE * NT))
        emsum_dve = em_tiles[list(DVE_LIST)].sum()
        in_maps.append({
            "hT": hT16[b],
            "wq": wq16,
            "bq_d": bq_dev,
            "wkv": np.ascontiguousarray(wkv_c).astype(f16),
            "bkv_d": np.ascontiguousarray(bkv_c).astype(f16),
            "mkt": np.ascontiguousarray(mkt_c).astype(f16),
            "mvt": np.ascontiguousarray(mvt_c).astype(f16),
            "em8_d": em8_dev.astype(f8),
            "em16_d": em16_dev.astype(f16),
            "mvts_d": mvts_dev.astype(f16),
            "emsum_d": np.array([[emsum_dve]], f32),
            "ones_d": ones_dev,
        })

    nc = _get_program()
    res = run_bass_kernel_spmd(nc, in_maps, core_ids=list(range(NCORES)),
                               trace=TRACE)
    LAST_RESULTS = res

    out = np.empty((B, S, NH, D), f32)
    for b in range(B):
        parts = res.results[b * NQUART]["out_d"].astype(f32).copy()
        for c in range(b * NQUART + 1, (b + 1) * NQUART):
            parts += res.results[c]["out_d"]
        num = parts[:, :D, :]                     # [12, 64, 768]
        den = parts[:, D, :]                      # [12, 768]
        ctxT = num / den[:, None, :]
        out[b] = ctxT.transpose(2, 0, 1)          # [768, 12, 64]
    g = (1.0 / (1.0 + np.exp(-gate))).reshape(1, 1, NH, 1)
    return (g * out).astype(f32)


# revision 27
# speedup vs baseline: 1.0380x; 1.0157x over previous
"""Trainium2 Bass kernel for BertInfiniSelfAttention — v3 design.

Sharding (8 cores): core c = (batch b = c//4, kv-quarter q = c%4).
Each core owns batch b and kv heads {3q, 3q+1, 3q+2} (KVL = 2304 of the
9216 concatenated kv positions), computes the full Q projection for its
batch, the K/V projections + memory matmuls for its 3 kv heads, then
flash-style partial attention for all 12 q heads against its local KV.
Host sums partial (numerator, denominator) over the 4 kv-quarters per
batch and divides.

v3: the ctx (probs @ V) matmuls run in fp8e4 DoubleRow mode — two kv
tiles (K=256) per pass at 1 column/cycle, 2x the fp16 rate.  Direct fp8
quantization of probs/values would exceed the error budget (softmax
averages 9216 near-uniform weights, so 3.6% rms fp8 noise survives into
the output), so the kernel uses a shift trick:

    ctx = sum_kv pr*vc = (sum_kv vc) + sum_kv (pr-1)*vc

The first term (vcsum) is computed exactly in fp16 via host-side row
sums of mvt (vcsum = mvtsum @ v, 18 N=1 matmuls) and added at flush
time; only the small residual pr' = exp(s)-1 (|pr'| ~ 0.17) and vc are
fp8, so quantization noise lands on a 6x smaller term.  The denominator
row works the same way: row 64 accumulates em*pr', and emsum (host
constant) is added at flush.

pr' production: qT is pre-scaled by 1/512 so scores psum holds s/64.
DVE tiles use a custom op (1+x)^64 - 1 (8 ALU stages); Act tiles
compute exp(64x) into fp16 staging and the otherwise-idle GpSimd engine
applies -1 with an fp8 cast.

Startup: wq/hT arrive in kt-major layout and are DMAed in 2-chunk
groups with per-group semaphores so the Q projection starts on group 0
(~12us) instead of after the full 2.4 MB (~20us).
"""

import numpy as np

B, S, H, NH, D = 2, 768, 768, 12, 64
P = 128
NCORES = 8
NQUART = 4              # kv quarters
HK = 3                  # kv heads per core
KVL = HK * S            # 2304 local kv
NKT = KVL // P          # 18 kv tiles

NT = S // P             # 6 s/H tiles
DP1 = D + 1
NPAIR = NH // 2         # 6 q-head pairs
SH = S // 2             # 384, flush s-half

# which kv tiles the Scalar engine exps (rest go to the Vector engine)
ACT_TILES = frozenset((0, 2, 4, 6, 8, 10, 12, 14, 16, 17))

_PROGRAM = None
TRACE = False
LAST_RESULTS = None


def _bank_pieces(lo, hi):
    """Split [lo,hi) free-dim range at 512-fp32 PSUM bank boundaries."""
    out = []
    while lo < hi:
        nxt = min(hi, (lo // 512 + 1) * 512)
        out.append((lo, nxt))
        lo = nxt
    return out


def _exp_ref(in0, in1, s0, s1, imm2):
    t = (in0 * s0 + 1.0).astype(np.float32)
    for _ in range(6):
        t = t * t
    return t


def _make_exp_op():
    from concourse import dve_ops as DO
    from concourse.dve_spec import Spec, Src0, C0, One, sq

    for o in DO.OPS:
        if o.name == "EXP_SQ64_ANT":
            return o
    body = sq(sq(sq(sq(sq(sq(Src0 * C0 + One))))))
    op = DO.DveOp(
        "EXP_SQ64_ANT",
        Spec(body=body, reference=_exp_ref),
        subdim=False,
        uops_sha={"v3": "52f44558ff295216", "v4": "63d0fb0e3de70366"},
    )
    DO.OPS.append(op)
    DO._SUB_OPCODE_FOR_NAME[op.name] = DO._CUSTOM_DVE_ROW_BASE + len(DO.OPS) - 1
    return op


def _build_program():
    from contextlib import ExitStack
    from itertools import zip_longest

    import concourse.bacc as bacc
    import concourse.mybir as mybir
    import concourse.tile as tile

    exp_op = _make_exp_op()

    F32 = mybir.dt.float32
    F16 = mybir.dt.float16
    F8 = mybir.dt.float8e4
    EXP = mybir.ActivationFunctionType.Exp
    IDT = mybir.ActivationFunctionType.Identity
    DR = mybir.MatmulPerfMode.DoubleRow
    ADD = mybir.AluOpType.add
    MULT = mybir.AluOpType.mult

    nc = bacc.Bacc("TRN2", target_bir_lowering=False, debug=False,
                   num_devices=NCORES)

    # hT / wq come in kt-major layout ([p, kt, c], pre-shuffled on the host)
    # so chunk-group DMAs are contiguous with 3 KiB per partition line
    hT = nc.declare_dram_parameter("hT", [P, NT * S], F16, isOutput=False)
    wq = nc.declare_dram_parameter("wq", [P, NT * H], F16, isOutput=False)
    bq_d = nc.declare_dram_parameter("bq_d", [P, NPAIR], F32, isOutput=False)
    wkv = nc.declare_dram_parameter("wkv", [H, 6 * D], F16, isOutput=False)
    bkv_d = nc.declare_dram_parameter("bkv_d", [1, 6 * D], F16, isOutput=False)
    mkt = nc.declare_dram_parameter("mkt", [S, KVL], F16, isOutput=False)
    mvt = nc.declare_dram_parameter("mvt", [S, KVL], F16, isOutput=False)
    em16_d = nc.declare_dram_parameter("em16_d", [P, NKT], F16,
                                       isOutput=False)
    ones_d = nc.declare_dram_parameter("ones_d", [1, S], F16, isOutput=False)
    out_d = nc.declare_dram_parameter("out_d", [NH, DP1, S], F32, isOutput=True)

    with tile.TileContext(nc) as tc, ExitStack() as ctx:
        const = ctx.enter_context(tc.tile_pool(name="const", bufs=1))

        qT = const.tile([P, NPAIR * S], F16, name="qT")
        kcT = const.tile([P, KVL], F16, name="kcT")
        vca = const.tile([P, NKT * DP1], F16, name="vca")
        ones = const.tile([1, S], F16, name="ones")
        bq_s = const.tile([P, NPAIR], F32, name="bq_s")
        bkv_s = const.tile([1, 6 * D], F16, name="bkv_s")
        act_warm = const.tile([1, 8], F16, name="act_warm")
        act_in = const.tile([1, 8], F16, name="act_in")

        nc.gpsimd.dma_start(ones[:], ones_d[:])
        nc.gpsimd.dma_start(bq_s[:], bq_d[:])
        nc.gpsimd.dma_start(bkv_s[:], bkv_d[:])
        # prefill the em (denominator) columns of vca straight from DRAM
        nc.gpsimd.dma_start(
            vca[:].rearrange("p (t c) -> p t c", c=DP1)[:, :, D:DP1],
            em16_d[:].rearrange("p (t c) -> p t c", c=1))
        # memset-sourced input for the exp-table warm: no DMA dependency, so
        # the scalar engine reaches its hT dma_starts without stalling
        nc.vector.memset(act_in[:], 1.0)

        # ---- long-lived inputs (kv3 + mvt live until vc is done) ----
        iov = ctx.enter_context(tc.tile_pool(name="iov", bufs=1))
        kv3 = iov.tile([P, NT * 6 * D], F16, name="kv3")
        mvt_s = iov.tile([P, NT * KVL], F16, name="mvt_s")

        with tc.tile_pool(name="iok", bufs=1) as iok:
            mkt_s = iok.tile([P, NT * KVL], F16, name="mkt_s")

            # ---- Phase A ----
            with tc.tile_pool(name="ioa", bufs=1) as ioa:
                wq_s = ioa.tile([P, NT * H], F16, name="wq_s")
                hT_s = ioa.tile([P, NT * S], F16, name="hT_s")
                wkv_s = ioa.tile([P, NT * 6 * D], F16, name="wkv_s")

                # DMA queues: sync = wq, wkv then outputs; scalar = hT
                # only (keeps the ACT instruction stream clean for phase-C
                # exps); gpsimd = consts, mkt, then mvt.  wq/hT arrive in
                # kt-major layout, issued as 2-chunk groups: contiguous
                # 3 KiB partition lines, with per-group completion
                # semaphores so the kt-chunked Q proj starts on group 0.
                for g in range(NT // 2):
                    nc.sync.dma_start(wq_s[:, 2 * g * H:2 * (g + 1) * H],
                                      wq[:, 2 * g * H:2 * (g + 1) * H])
                    nc.scalar.dma_start(hT_s[:, 2 * g * S:2 * (g + 1) * S],
                                        hT[:, 2 * g * S:2 * (g + 1) * S])
                for kt in range(NT):
                    nc.sync.dma_start(wkv_s[:, kt * 6 * D:(kt + 1) * 6 * D],
                                      wkv[kt * P:(kt + 1) * P, :])
                # warm the exp table (ACT_TABLE_LOAD ~1.3us) during phase A
                nc.scalar.activation(act_warm[:], act_in[:], EXP, scale=0.125)
                for h in range(HK):
                    for st in range(NT):
                        nc.gpsimd.dma_start(
                            mkt_s[:, st * KVL + h * S: st * KVL + (h + 1) * S],
                            mkt[st * P:(st + 1) * P, h * S:(h + 1) * S])
                for h in range(HK):
                    for st in range(NT):
                        nc.gpsimd.dma_start(
                            mvt_s[:, st * KVL + h * S: st * KVL + (h + 1) * S],
                            mvt[st * P:(st + 1) * P, h * S:(h + 1) * S])

                # Q projection (pair-outer) + K/V projection
                with tc.tile_pool(name="aps", bufs=2, space="PSUM") as aps:
                    for t in range(NPAIR):
                        q_ps = aps.tile([P, S], F32, name="q_ps", tag="q_ps")
                        for lo, hi in _bank_pieces(0, S):
                            for kt in range(NT):
                                nc.tensor.matmul(
                                    q_ps[:, lo:hi],
                                    wq_s[:, kt * H + t * P: kt * H + (t + 1) * P],
                                    hT_s[:, kt * S + lo: kt * S + hi],
                                    start=(kt == 0), stop=(kt == NT - 1))
                        if t % 2 == 0:
                            nc.vector.tensor_scalar_add(
                                qT[:, t * S:(t + 1) * S], q_ps[:],
                                bq_s[:, t:t + 1])
                        else:
                            nc.scalar.activation(
                                qT[:, t * S:(t + 1) * S], q_ps[:], IDT,
                                bias=bq_s[:, t:t + 1])

                    for st in range(NT):
                        kv_ps = aps.tile([P, 6 * D], F32, name="kv_ps",
                                         tag="kv_ps")
                        for kt in range(NT):
                            nc.tensor.matmul(
                                kv_ps[:],
                                hT_s[:, kt * S + st * P: kt * S + (st + 1) * P],
                                wkv_s[:, kt * 6 * D:(kt + 1) * 6 * D],
                                start=(kt == 0), stop=False)
                        nc.tensor.matmul(kv_ps[:], ones[:, 0:P], bkv_s[:],
                                         start=False, stop=True)
                        nc.vector.tensor_copy(
                            kv3[:, st * 6 * D:(st + 1) * 6 * D], kv_ps[:])

            # ---- Phase B: kc (kv-duplicated halves, concurrent col pairs)
            with tc.tile_pool(name="kcps", bufs=2, space="PSUM") as kcps:
                for h in range(HK):
                    kc_ps = kcps.tile([P, S], F32, name="kc_ps", tag="kc_ps")
                    for lo, hi in _bank_pieces(0, S):
                        for st in range(NT):
                            lhsT = kv3[:, st * 6 * D + h * D:
                                       st * 6 * D + (h + 1) * D]
                            rhs = mkt_s[:, st * KVL + h * S + lo:
                                        st * KVL + h * S + hi]
                            nc.tensor.matmul(
                                kc_ps[0:D, lo:hi], lhsT, rhs,
                                start=(st == 0), stop=(st == NT - 1))
                            nc.tensor.matmul(
                                kc_ps[D:P, lo:hi], lhsT, rhs,
                                start=(st == 0), stop=(st == NT - 1),
                                tile_position=(0, D))
                    nc.vector.tensor_copy(kcT[:, h * S:(h + 1) * S], kc_ps[:])


        # ---- Phase C ----
        prp = ctx.enter_context(tc.tile_pool(name="prp", bufs=38))
        scps = ctx.enter_context(tc.tile_pool(name="scps", bufs=2,
                                              space="PSUM"))
        stg = ctx.enter_context(tc.tile_pool(name="stg", bufs=4))

        def emit_scores(p, t):
            """Concurrent row-tiled pair: even head -> cols 0:768, odd
            head -> cols 768:1536 of a [128, 1536] psum tile."""
            sc = scps.tile([P, 2 * S], F32, name="sc", tag="sc")
            kc_lo = kcT[0:D, t * P:(t + 1) * P]
            kc_hi = kcT[D:P, t * P:(t + 1) * P]
            for pa, pb in zip_longest(_bank_pieces(0, S),
                                      _bank_pieces(S, 2 * S)):
                if pa is not None:
                    lo, hi = pa
                    nc.tensor.matmul(sc[:, lo:hi], kc_lo,
                                     qT[0:D, p * S + lo: p * S + hi],
                                     start=True, stop=True)
                if pb is not None:
                    lob, hib = pb
                    nc.tensor.matmul(
                        sc[:, lob:hib], kc_hi,
                        qT[D:P, p * S + lob - S: p * S + hib - S],
                        start=True, stop=True)
            pr = prp.tile([P, 2 * S], F16, name="pr", tag="pr")
            if t in ACT_TILES:
                nc.scalar.activation(pr[:], sc[:], EXP, scale=0.125)
            else:
                nc.vector._custom_dve(exp_op, out=pr[:], in0=sc[:],
                                      s0=0.125 / 64)
            return pr

        def emit_ctx(t, pr, half, ps, first, last):
            base = half * S
            for lo, hi in _bank_pieces(0, S):
                nc.tensor.matmul(
                    ps[:, lo:hi], vca[:, t * DP1:(t + 1) * DP1],
                    pr[:, base + lo: base + hi],
                    start=first, stop=last)

        def flush_ctx(ps, head):
            st_t = stg.tile([DP1, S], F32, name="st_t", tag="st")
            nc.vector.tensor_copy(st_t[:, 0:SH], ps[:, 0:SH])
            nc.scalar.copy(st_t[:, SH:S], ps[:, SH:S])
            nc.sync.dma_start(out_d[head], st_t[:])

        def emit_vc(t):
            h = t // NT
            vc_ps = vcps.tile([P, D], F32, name="vc_ps", tag="vc")
            for st in range(NT):
                nc.tensor.matmul(
                    vc_ps[:],
                    mvt_s[:, st * KVL + t * P: st * KVL + (t + 1) * P],
                    kv3[:, st * 6 * D + (HK + h) * D:
                        st * 6 * D + (HK + h + 1) * D],
                    start=(st == 0), stop=(st == NT - 1))
            nc.vector.tensor_copy(vca[:, t * DP1: t * DP1 + D], vc_ps[:])

        # sweep 0: scores/exp for pair 0 with the vc matmuls interleaved
        # (PE filler under the engine-paced exp window)
        prev = []
        with tc.tile_pool(name="vcps", bufs=2, space="PSUM") as vcps:
            for t in range(NKT):
                prev.append(emit_scores(0, t))
                emit_vc(t)

        # sweeps 1..6: scores/exp for pair p + deferred ctx chains for
        # pair p-1 (A then B) sharing one psum accumulator
        with tc.tile_pool(name="ctxps", bufs=1, space="PSUM") as ctxps:
            for p in range(1, NPAIR + 1):
                cur = []
                ps = None
                for t in range(NKT):
                    if t in (0, 9) and p < NPAIR:
                        # chain-transition slots: scores first so the PE
                        # isn't head-of-line blocked on the prior flush
                        cur.append(emit_scores(p, t))
                    if t < 9:   # ctx A: tiles 2t, 2t+1
                        if t == 0:
                            ps = ctxps.tile([DP1, S], F32, name="ctx",
                                            tag="ctx")
                        emit_ctx(2 * t, prev[2 * t], 0, ps,
                                 first=(t == 0), last=False)
                        emit_ctx(2 * t + 1, prev[2 * t + 1], 0, ps,
                                 first=False, last=(t == 8))
                        if t == 8:
                            flush_ctx(ps, 2 * (p - 1))
                    else:       # ctx B: tiles 2(t-9), 2(t-9)+1
                        tb = t - 9
                        if tb == 0:
                            ps = ctxps.tile([DP1, S], F32, name="ctx",
                                            tag="ctx")
                        emit_ctx(2 * tb, prev[2 * tb], 1, ps,
                                 first=(tb == 0), last=False)
                        emit_ctx(2 * tb + 1, prev[2 * tb + 1], 1, ps,
                                 first=False, last=(tb == 8))
                        if tb == 8:
                            flush_ctx(ps, 2 * (p - 1) + 1)
                    if p < NPAIR and t not in (0, 9):
                        cur.append(emit_scores(p, t))
                prev = cur

    nc.compile()
    return nc


def _get_program():
    global _PROGRAM
    if _PROGRAM is None:
        _PROGRAM = _build_program()
    return _PROGRAM


def kernel(hidden_states, attention_mask, Wq, bq, Wk, bk, Wv, bv, gate,
           mem_keys, mem_values):
    from concourse.bass_utils import run_bass_kernel_spmd

    global LAST_RESULTS

    f32, f16 = np.float32, np.float16
    hidden_states = np.asarray(hidden_states, f32)
    attention_mask = np.asarray(attention_mask, f32)
    Wq = np.asarray(Wq, f32)
    bq = np.asarray(bq, f32)
    Wk = np.asarray(Wk, f32)
    bk = np.asarray(bk, f32)
    Wv = np.asarray(Wv, f32)
    bv = np.asarray(bv, f32)
    gate = np.asarray(gate, f32)
    mem_keys = np.asarray(mem_keys, f32)
    mem_values = np.asarray(mem_values, f32)

    # kt-major device layouts: x_kt[p, kt*C + c] = x[kt*128 + p, c]
    hT16 = [np.ascontiguousarray(
                hidden_states[b].T.reshape(NT, P, S).transpose(1, 0, 2)
                .reshape(P, NT * S)).astype(f16)
            for b in range(B)]
    wq16 = np.ascontiguousarray(
        Wq.reshape(NT, P, H).transpose(1, 0, 2).reshape(P, NT * H)
    ).astype(f16)
    bq_dev = np.ascontiguousarray(
        bq.reshape(NPAIR, 2, D).transpose(1, 2, 0).reshape(P, NPAIR))
    em_full = np.exp(attention_mask.reshape(B, NH * S)).astype(f32)
    ones_dev = np.ones((1, S), f16)

    in_maps = []
    for c in range(NCORES):
        b, quart = c // NQUART, c % NQUART
        heads = [HK * quart + j for j in range(HK)]
        wkv_c = np.concatenate(
            [Wk[:, h * D:(h + 1) * D] for h in heads]
            + [Wv[:, h * D:(h + 1) * D] for h in heads], axis=1)
        bkv_c = np.concatenate(
            [bk[h * D:(h + 1) * D] for h in heads]
            + [bv[h * D:(h + 1) * D] for h in heads])[None, :]
        # mkt[s, h_local*768+kv] = mem_keys[heads[h_local], kv, s]
        mkt_c = mem_keys[heads].transpose(2, 0, 1).reshape(S, KVL)
        em_c = em_full[b, quart * KVL:(quart + 1) * KVL]
        mvt_c = (mem_values[heads].transpose(2, 0, 1).reshape(S, KVL)
                 * em_c[None, :])
        em_dev = np.ascontiguousarray(em_c.reshape(NKT, P).T)
        in_maps.append({
            "hT": hT16[b],
            "wq": wq16,
            "bq_d": bq_dev,
            "wkv": np.ascontiguousarray(wkv_c).astype(f16),
            "bkv_d": np.ascontiguousarray(bkv_c).astype(f16),
            "mkt": np.ascontiguousarray(mkt_c).astype(f16),
            "mvt": np.ascontiguousarray(mvt_c).astype(f16),
            "em16_d": em_dev.astype(f16),
            "ones_d": ones_dev,
        })

    nc = _get_program()
    res = run_bass_kernel_spmd(nc, in_maps, core_ids=list(range(NCORES)),
                               trace=TRACE)
    LAST_RESULTS = res

    out = np.empty((B, S, NH, D), f32)
    for b in range(B):
        parts = res.results[b * NQUART]["out_d"].astype(f32).copy()
        for c in range(b * NQUART + 1, (b + 1) * NQUART):
            parts += res.results[c]["out_d"]
        num = parts[:, :D, :]                     # [12, 64, 768]
        den = parts[:, D, :]                      # [12, 768]
        ctxT = num / den[:, None, :]
        out[b] = ctxT.transpose(2, 0, 1)          # [768, 12, 64]
    g = (1.0 / (1.0 + np.exp(-gate))).reshape(1, 1, NH, 1)
    return (g * out).astype(f32)
